# revision 40
# baseline (speedup 1.0000x reference)
"""Trainium2 Bass kernel for nn_Attention_54614804136573 (topk_masking).

Sharding: 8 cores = 4 batches x 2 head-groups (8 heads each). Each core gets
its batch's full x (columns rotated so its own 8 head-chunks come first),
computes the token-importance mask redundantly, runs its 8 heads of attention,
and produces a partial to_out product over its 1024-wide d-slice for all 2048
output channels. The host sums the two partials per batch and adds bo'
(bo with the V-bias term folded in).

Key structure vs the previous version:
- x arrives as bf16 and is transposed into SBUF chunk-wise by the DMA XBAR
  (dma_start_transpose), removing all PE transposes and PSUM copies for x.
- V is projected directly into its PV-ready [token, channel] layout by using
  the x chunk as the stationary operand (out = xT_chunk.T @ WvT), removing
  the separate V transpose pass. The V bias is exactly handled outside the
  kernel: a = u/dn + bv*(S_pm/dn) with S_pm ~= dn + (sum(mask)-1024), so the
  per-head bias folds into an Act bias (k*bv) plus a host-side bo term.
- The softmax denominator is an M=1 ones-matmul accumulated over all 8
  j-tiles into a [1, N] PSUM row (ones = 1/32 so the normalized output is
  pre-scaled by 32 for fp8).
- to_out runs in fp8 DoubleRow (K=256/instr at 0.5 cycles/row) with an
  error-compensated hi+lo split of both Wo (host side, x64) and the
  attention output (device side, x32): w*a ~= w_hi*a_hi + w_hi*a_lo +
  w_lo*a_hi, 12 DR matmuls per (oc, half) instead of 16 bf16 matmuls.
"""

import sys

sys.path.insert(0, "/opt/trn_rl_repo")

import numpy as np
import ml_dtypes

import concourse.mybir as mybir
import concourse.tile as tile
from concourse import bacc, bass_utils
from concourse.tile import add_dep_helper

B = 4
N = 1024
C = 128
D = 2048
NCHUNK = 16  # d-chunks of 128 (= patch positions = heads)
HPC = 8  # heads per core
MASK_NUM = 25
SCALE = 64.0 ** -0.5  # 0.125

F32 = mybir.dt.float32
BF16 = mybir.dt.bfloat16
FP8 = mybir.dt.float8e4
U32 = mybir.dt.uint32
Exp = mybir.ActivationFunctionType.Exp
Ident = mybir.ActivationFunctionType.Identity
Ln = mybir.ActivationFunctionType.Ln
DR = mybir.MatmulPerfMode.DoubleRow
NEG_BIG = -1e30

WO_SCALE = 64.0  # host-side Wo prescale before fp8 split
A_SCALE = 32.0   # device-side attention-out prescale (via ones = 1/32)


def _body(tc, xc, wq_d, bq_d, wtc_d, wo_hi_d, wo_lo_d, outT_d):
    nc = tc.nc
    dscr = nc.dram_tensor("dscr", (HPC, N), F32, kind="Internal").ap()

    with (
        tc.tile_pool(name="consts", bufs=1) as consts,
        tc.tile_pool(name="persist", bufs=1) as persist,
    ):
        # ---- constants ----
        ones32 = consts.tile([128, 1], BF16)
        nc.vector.memset(ones32, 1.0 / A_SCALE)
        one_f32 = consts.tile([1, 1], F32)
        nc.vector.memset(one_f32, 1.0)
        ones128_f32 = consts.tile([1, 128], F32)
        nc.vector.memset(ones128_f32, 1.0)
        ones128_bf = consts.tile([1, 128], BF16)
        nc.vector.memset(ones128_bf, 1.0)

        # ---- persistent activations ----
        qT = persist.tile([128, HPC, N], BF16)      # [c', h, n] 2 MB
        kT = persist.tile([128, HPC, N], BF16)      # 2 MB
        vnat = persist.tile([128, HPC, 8, C], BF16)  # [j, h, jt, c] 2 MB
        outT_hi = persist.tile([128, HPC, N], FP8)  # 32*(a - bv) hi split
        outT_lo = persist.tile([128, HPC, N], FP8)
        wo_hi = persist.tile([128, HPC, D], FP8)    # [d, h-chunk, o] 2 MB
        wo_lo = persist.tile([128, HPC, D], FP8)
        mask_col = persist.tile([128, 8], F32)
        scale_col = persist.tile([128, 8], F32)
        lnm_col = persist.tile([128, 8], F32)       # ln(mask) exp bias
        kbv = persist.tile([128, 1], F32)           # (sum(m)-1024) * bv
        ksc32 = persist.tile([1, 1], F32)           # (sum(m)-1024)/32

        # ============ phase 1: x transpose-in, logits, mask, QKV ===========
        with (
            tc.tile_pool(name="ph1big", bufs=1) as ph1big,
            tc.tile_pool(name="mrows", bufs=1) as mrows,
            tc.tile_pool(name="mm_psum", bufs=2, space="PSUM") as mm_psum,
            tc.tile_pool(name="v_psum", bufs=2, space="PSUM") as v_psum,
        ):
            # packed weight loads first (tiny; scalar queue)
            wqkv_sb = consts.tile([C, 3 * C], BF16)
            nc.scalar.dma_start(out=wqkv_sb, in_=wq_d)
            wq_sb = wqkv_sb[:, 0:C]
            wk_sb = wqkv_sb[:, C : 2 * C]
            wv_sb = wqkv_sb[:, 2 * C : 3 * C]
            bqkv_sb = consts.tile([C, 3], F32)
            nc.scalar.dma_start(out=bqkv_sb, in_=bq_d)
            bq_sb = bqkv_sb[:, 0:1]
            bk_sb = bqkv_sb[:, 1:2]
            bv_sb = bqkv_sb[:, 2:3]
            wtc_sb = consts.tile([C, 1], BF16)
            nc.scalar.dma_start(out=wtc_sb, in_=wtc_d)

            # x transposed in by the DMA XBAR in 8 strided sweeps (pipelined
            # so logits can chase them). xc rows viewed as [(n k), c] with
            # row-stride 256B give layout xT[c, n, k] (k fastest on free).
            NP = 8
            PR = N // NP  # 128 tokens per piece
            xT = ph1big.tile([128, N, NCHUNK], BF16)  # [c, n, k] 4 MB
            xc_rows = xc.rearrange("n (k c) -> (n k) c", c=128)
            tp_insts = []
            for p in range(NP):
                hr = slice(p * PR * NCHUNK, (p + 1) * PR * NCHUNK)
                tp_insts.append(nc.sync.dma_start_transpose(
                    out=xT[:, p * PR : (p + 1) * PR, :],
                    in_=xc_rows[hr, :],
                ))

            # Wo hi/lo splits: one big DMA each, after the x transposes.
            for wo_sb, wo_src, dep in (
                (wo_hi, wo_hi_d, tp_insts[-2]),
                (wo_lo, wo_lo_d, tp_insts[-1]),
            ):
                wi = nc.gpsimd.dma_start(
                    out=wo_sb,
                    in_=wo_src.rearrange("(h p) o -> p h o", p=128),
                )
                add_dep_helper(wi.ins, dep.ins, sync=True, reason="wo after x")

            with tc.tile_pool(name="lg_psum", bufs=1, space="PSUM") as lg_psum:
                # logits[n] = sum_k xT[:, n, k] . wtc   (wtc = (Wl@Wq)/16),
                # one accumulation group per transpose piece so PE starts as
                # soon as the first piece lands.
                lg = lg_psum.tile([1, N], F32)
                negrow = mrows.tile([1, N], F32)
                for p in range(NP):
                    for k in range(NCHUNK):
                        nc.tensor.matmul(
                            lg[:, p * PR : (p + 1) * PR],
                            wtc_sb,
                            xT[:, p * PR : (p + 1) * PR, k],
                            start=(k == 0),
                            stop=(k == NCHUNK - 1),
                        )
                    # negate per piece: runs on DVE while later pieces land
                    nc.vector.tensor_scalar_mul(
                        negrow[:, p * PR : (p + 1) * PR],
                        lg[:, p * PR : (p + 1) * PR], -1.0,
                    )

                # ---- mask: softmax over tokens, snap all but 25 smallest
                # to 1. DVE runs the serial top-k; Act computes the softmax
                # normalization in parallel.
                m8 = mrows.tile([1, 8], F32)
                for _ in range(3):
                    nc.vector.max(out=m8, in_=negrow)
                    nc.vector.match_replace(
                        out=negrow, in_to_replace=m8, in_values=negrow,
                        imm_value=NEG_BIG,
                    )
                nc.vector.max(out=m8, in_=negrow)  # 25th largest of -L
                thrneg = mrows.tile([1, 1], F32)
                nc.vector.tensor_scalar_mul(thrneg, m8[:, 0:1], -1.0)
                urow = mrows.tile([1, N], F32)
                ssum = mrows.tile([1, 1], F32)
                nc.scalar.activation(
                    out=urow, in_=lg, func=Exp, accum_out=ssum
                )
                srecip = mrows.tile([1, 1], F32)
                nc.vector.reciprocal(srecip, ssum)
                smrow = mrows.tile([1, N], F32)
                nc.scalar.activation(
                    out=smrow, in_=urow, func=Ident, scale=srecip
                )
                sel = mrows.tile([1, N], U32)
                nc.vector.tensor_scalar(
                    sel, lg, thrneg, None, op0=mybir.AluOpType.is_gt
                )
                onesrow = mrows.tile([1, N], F32)
                nc.vector.memset(onesrow, 1.0)
                nc.vector.copy_predicated(smrow, sel, onesrow)
                # k = sum(mask) - 1024 (~ -25 + tiny): V-bias fold + dn fixup
                msum = mrows.tile([1, 1], F32)
                mdummy = mrows.tile([1, N], F32)
                nc.vector.tensor_scalar(
                    mdummy, smrow, 1.0, 0.0, op0=mybir.AluOpType.mult,
                    op1=mybir.AluOpType.add, accum_out=msum,
                )
                ksc = mrows.tile([1, 1], F32)
                nc.vector.tensor_scalar_add(ksc, msum, -float(N))
                nc.vector.tensor_scalar_mul(ksc32, ksc, 1.0 / A_SCALE)

            # ---- Q/K/V projections, interleaved per head -----------------
            # Q/K produce transposed layouts [c', h, n]; V goes directly to
            # its PV-ready [token, channel] layout (x chunk as stationary).
            # V bias is handled via kbv + host bo fold; the V mask lives in
            # the exp bias (lnm_col), so V copies have no mask dependency.
            # Act paces the Q/K bias-moves; V copies go to DVE (idle once
            # the mask chain drains) except the last heads on Act.
            for h in range(HPC):
                for jtg in range(2):
                    vp = v_psum.tile([128, 4, C], F32)
                    for dj in range(4):
                        jt = jtg * 4 + dj
                        nc.tensor.matmul(
                            vp[:, dj, :],
                            xT[:, jt * 128 : (jt + 1) * 128, h],
                            wv_sb,
                            start=True,
                            stop=True,
                        )
                    dst = vnat[:, h, jtg * 4 : (jtg + 1) * 4, :]
                    if h < 6:
                        nc.vector.tensor_copy(dst, vp)
                    else:
                        nc.scalar.activation(out=dst, in_=vp, func=Ident)
                for w_sb, b_sb, dstT in ((wq_sb, bq_sb, qT), (wk_sb, bk_sb, kT)):
                    pp = mm_psum.tile([128, N], F32)
                    for half in range(2):
                        nc.tensor.matmul(
                            pp[:, half * 512 : (half + 1) * 512],
                            w_sb,
                            xT[:, half * 512 : (half + 1) * 512, h],
                            start=True,
                            stop=True,
                        )
                    nc.scalar.activation(
                        out=dstT[:, h, :], in_=pp, func=Ident, bias=b_sb
                    )

            # ---- mask row -> [128, 8] columns via tiny PE transposes ------
            # (plus a [128,1] broadcast of k), replacing two DRAM bounces.
            with tc.tile_pool(name="mc_psum", bufs=1, space="PSUM") as mc_psum:
                mcol_ps = mc_psum.tile([128, 9], F32)
                for t in range(8):
                    nc.tensor.transpose(
                        mcol_ps[:, t : t + 1],
                        smrow[:, t * 128 : (t + 1) * 128],
                        one_f32,
                    )
                nc.tensor.matmul(
                    mcol_ps[:, 8:9], ones128_f32, ksc, start=True, stop=True
                )
                nc.vector.tensor_copy(mask_col, mcol_ps[:, 0:8])
                nc.vector.tensor_scalar_mul(scale_col, mask_col, SCALE)
                nc.scalar.activation(out=lnm_col, in_=mask_col, func=Ln)
                nc.vector.tensor_mul(kbv, mcol_ps[:, 8:9], bv_sb)

        # ================= phase 2: attention ==============================
        # Pool open order places dn/ot on the earliest-freed phase-1 banks.
        with (
            tc.tile_pool(name="pexp", bufs=2) as pexp_pool,
            tc.tile_pool(name="dvp", bufs=2) as dvp,
            tc.tile_pool(name="dn_psum", bufs=1, space="PSUM") as dn_psum,
            tc.tile_pool(name="ot_psum", bufs=1, space="PSUM") as ot_psum,
            tc.tile_pool(name="st_psum", bufs=2, space="PSUM") as st_psum,
        ):
            for h in range(HPC):
                ot = ot_psum.tile([128, N], F32, tag="ot")
                dn = dn_psum.tile([1, N], F32, tag="dn")
                pexp = pexp_pool.tile([128, 8, N], BF16, tag="pexp")

                def emit_pvdn(jt, ot=ot, dn=dn, pexp=pexp, h=h):
                    for half in range(2):
                        sl = slice(half * 512, (half + 1) * 512)
                        nc.tensor.matmul(
                            ot[:, sl],
                            vnat[:, h, jt, :],
                            pexp[:, jt, sl],
                            start=(jt == 0),
                            stop=(jt == 7),
                        )
                        nc.tensor.matmul(
                            dn[:, sl],
                            ones32,
                            pexp[:, jt, sl],
                            start=(jt == 0),
                            stop=(jt == 7),
                        )

                pending = None
                for jt in range(8):
                    st = st_psum.tile([128, N], F32, tag="st")
                    for half in range(2):
                        nc.tensor.matmul(
                            st[:, half * 512 : (half + 1) * 512],
                            kT[:, h, jt * 128 : (jt + 1) * 128],
                            qT[:, h, half * 512 : (half + 1) * 512],
                            start=True,
                            stop=True,
                        )
                    # exp split per half: halves the S->PV latency so the
                    # queued PE work covers the Act round-trip
                    for half in range(2):
                        sl = slice(half * 512, (half + 1) * 512)
                        nc.scalar.activation(
                            out=pexp[:, jt, sl], in_=st[:, sl], func=Exp,
                            scale=scale_col[:, jt : jt + 1],
                            bias=lnm_col[:, jt : jt + 1],
                        )
                    if pending is not None:
                        emit_pvdn(pending)
                    pending = jt
                emit_pvdn(pending)

                # drain: otsb = u + k*bv (DVE per-partition add, frees ot);
                # dn' holds sum_j m_j p_j / 32, true dn/32 = dn' - k/32 (the
                # 25 masked tokens have p ~= 1); tmp = otsb * (32/dn);
                # hi/lo fp8 split of tmp.
                otsb = dvp.tile([128, N], BF16, tag="otsb")
                if h < HPC - 1:
                    nc.vector.tensor_scalar(
                        otsb, ot, kbv, None, op0=mybir.AluOpType.add
                    )
                else:
                    # last head: Act is idle here; keep DVE free for the
                    # to_out-gating dnadj/recip/tmp/hi/lo chain
                    nc.scalar.activation(
                        out=otsb, in_=ot, func=Ident, bias=kbv
                    )
                dnadj = dvp.tile([1, N], F32, tag="dnadj")
                nc.vector.tensor_scalar(
                    dnadj, dn, ksc32, None, op0=mybir.AluOpType.subtract
                )
                if h < HPC - 1:
                    rrow = dvp.tile([1, N], F32, tag="rrow")
                    nc.vector.reciprocal(rrow, dnadj)
                    w_i = nc.sync.dma_start(out=dscr[h, :], in_=rrow)
                    rb = dvp.tile([128, N], F32, tag="rb")
                    r_i = nc.sync.dma_start(
                        out=rb, in_=dscr[h, :].partition_broadcast(128)
                    )
                    add_dep_helper(r_i.ins, w_i.ins, sync=True,
                                   reason="recip RAW")
                else:
                    # last head gates to_out: broadcast 1/dn across partitions
                    # with a K=1 matmul into the freed ot slot instead of the
                    # (slower) DRAM round-trip.
                    rrow_bf = dvp.tile([1, N], BF16, tag="rrowbf")
                    with nc.allow_low_precision(
                        reason="1/dn broadcast row; 0.4% relative is fine"
                    ):
                        nc.vector.reciprocal(rrow_bf, dnadj)
                    rb = ot_psum.tile([128, N], F32, tag="ot")
                    for half in range(2):
                        sl = slice(half * 512, (half + 1) * 512)
                        nc.tensor.matmul(
                            rb[:, sl], ones128_bf, rrow_bf[:, sl],
                            start=True, stop=True,
                        )
                tmp = dvp.tile([128, N], BF16, tag="tmp")
                nc.vector.tensor_mul(tmp, otsb, rb)
                nc.vector.tensor_copy(outT_hi[:, h, :], tmp)
                nc.vector.tensor_sub(
                    outT_lo[:, h, :], tmp, outT_hi[:, h, :]
                )

            # ============= phase 4: to_out partial (fp8 DoubleRow) =========
            # fo shares the st_psum slots (same shape) so Wo accumulation can
            # begin as soon as the last exp frees an ST slot.
            with tc.tile_pool(name="fout", bufs=3) as fout_pool:
                PRODUCTS = ((wo_hi, outT_hi), (wo_hi, outT_lo), (wo_lo, outT_hi))

                def finish_oc(oc, fo):
                    fout = fout_pool.tile([128, N], BF16)
                    eng = nc.sync if oc % 2 == 0 else nc.gpsimd
                    if oc < 15:
                        nc.vector.tensor_scalar_mul(
                            fout, fo, 1.0 / (WO_SCALE * A_SCALE)
                        )
                        eng.dma_start(
                            out=outT_d[oc * 128 : (oc + 1) * 128, :], in_=fout
                        )
                    else:
                        # final oc: drain per half so the tail DMA covers
                        # only 512 columns
                        for sh in range(2):
                            ssl = slice(sh * 512, (sh + 1) * 512)
                            nc.vector.tensor_scalar_mul(
                                fout[:, ssl], fo[:, ssl],
                                1.0 / (WO_SCALE * A_SCALE),
                            )
                            eng = nc.sync if sh == 0 else nc.gpsimd
                            eng.dma_start(
                                out=outT_d[oc * 128 : (oc + 1) * 128, ssl],
                                in_=fout[:, ssl],
                            )

                pending_oc = None
                for oc in range(16):
                    fo = st_psum.tile([128, N], F32, tag="st")
                    for half in range(2):
                        sl = slice(half * 512, (half + 1) * 512)
                        # t=3 (heads 6,7) last in every product so the first
                        # 9 instructions don't wait on head 7's drain chain
                        steps = [(w, a, t) for t in (0, 1, 2)
                                 for (w, a) in PRODUCTS]
                        steps += [(w, a, 3) for (w, a) in PRODUCTS]
                        for i, (wsp, asp, t) in enumerate(steps):
                            nc.tensor.matmul(
                                fo[:, sl],
                                wsp[:, 2 * t : 2 * t + 2,
                                    oc * 128 : (oc + 1) * 128],
                                asp[:, 2 * t : 2 * t + 2, sl],
                                start=(i == 0),
                                stop=(i == len(steps) - 1),
                                perf_mode=DR,
                            )
                    if pending_oc is not None:
                        finish_oc(*pending_oc)
                    pending_oc = (oc, fo)
                finish_oc(*pending_oc)


_CACHE = {}


def _get_module():
    if "nc" in _CACHE:
        return _CACHE["nc"]
    nc = bacc.Bacc("TRN2", target_bir_lowering=False, debug=False, num_devices=8)
    xc = nc.dram_tensor("xc", (N, D), BF16, kind="ExternalInput").ap()
    wq_d = nc.dram_tensor("wqkvT", (C, 3 * C), BF16, kind="ExternalInput").ap()
    bq_d = nc.dram_tensor("bqkv", (C, 3), F32, kind="ExternalInput").ap()
    wtc_d = nc.dram_tensor("wtc", (C, 1), BF16, kind="ExternalInput").ap()
    wo_hi_d = nc.dram_tensor("woHi", (HPC * C, D), FP8, kind="ExternalInput").ap()
    wo_lo_d = nc.dram_tensor("woLo", (HPC * C, D), FP8, kind="ExternalInput").ap()
    outT_d = nc.dram_tensor("outT", (D, N), BF16, kind="ExternalOutput").ap()

    with tile.TileContext(nc) as tc:
        _body(tc, xc, wq_d, bq_d, wtc_d, wo_hi_d, wo_lo_d, outT_d)
    nc.compile()
    _CACHE["nc"] = nc
    return nc


def make_in_maps(x, Wq, bq, Wk, bk, Wv, bv, Wl, bl, Wo, bo):
    bf16 = ml_dtypes.bfloat16
    fp8 = ml_dtypes.float8_e4m3
    x = np.ascontiguousarray(np.asarray(x, np.float32))
    Wq = np.asarray(Wq, np.float32)
    Wk = np.asarray(Wk, np.float32)
    Wv = np.asarray(Wv, np.float32)
    Wl = np.asarray(Wl, np.float32)
    Wo = np.asarray(Wo, np.float32)
    we = (Wl[0] @ Wq) / float(NCHUNK)  # (128,) logits weight per chunk
    common = {
        "wqkvT": np.ascontiguousarray(
            np.concatenate([Wq.T, Wk.T, Wv.T], axis=1)
        ).astype(bf16),
        "bqkv": np.ascontiguousarray(np.stack(
            [np.asarray(bq, np.float32), np.asarray(bk, np.float32),
             np.asarray(bv, np.float32)], axis=1
        )),
        "wtc": we.astype(bf16).reshape(C, 1),
    }
    woT = np.ascontiguousarray(Wo.T) * WO_SCALE  # (d, o), prescaled
    in_maps = []
    halves = {}
    for g in range(2):
        wh = woT[g * 1024 : (g + 1) * 1024, :]
        hi = wh.astype(fp8)
        lo = (wh - hi.astype(np.float32)).astype(fp8)
        halves[g] = (np.ascontiguousarray(hi), np.ascontiguousarray(lo))
    for core in range(8):
        b, g = divmod(core, 2)
        xb = x[b]
        xcore = xb if g == 0 else np.concatenate(
            [xb[:, 1024:], xb[:, :1024]], axis=1
        )
        in_maps.append({
            "xc": np.ascontiguousarray(xcore.astype(bf16)),
            "woHi": halves[g][0],
            "woLo": halves[g][1],
            **common,
        })
    return in_maps


def run_spmd(in_maps, trace=False, **kw):
    nc = _get_module()
    return bass_utils.run_bass_kernel_spmd(
        nc, in_maps, core_ids=list(range(8)), trace=trace, **kw
    )


def gather(results, Wo, bv, bo):
    Wo = np.asarray(Wo, np.float32)
    bv = np.asarray(bv, np.float32)
    bo = np.asarray(bo, np.float32)
    # a = a_tilde + bv per head-channel: fold bv through Wo into the bias.
    bo_eff = bo + np.tile(bv, NCHUNK) @ Wo.T
    out = np.empty((B, N, D), np.float32)
    for b in range(B):
        p0 = results[2 * b]["outT"].astype(np.float32).T
        p1 = results[2 * b + 1]["outT"].astype(np.float32).T
        out[b] = p0 + p1 + bo_eff
    return out


def kernel(x, Wq, bq, Wk, bk, Wv, bv, Wl, bl, Wo, bo, stage=None, **_unused):
    in_maps = make_in_maps(x, Wq, bq, Wk, bk, Wv, bv, Wl, bl, Wo, bo)
    try:
        res = run_spmd(in_maps)
    except Exception:
        # transient device/runtime hiccup: retry once after a short pause
        import time as _time

        _time.sleep(2.0)
        res = run_spmd(in_maps)
    return gather(res.results, Wo, bv, bo)


# revision 42
# speedup vs baseline: 1.0181x; 1.0181x over previous
"""Trainium2 Bass kernel for nn_Attention_54614804136573 (topk_masking).

Sharding: 8 cores = 4 batches x 2 head-groups (8 heads each). Each core gets
its batch's full x (columns rotated so its own 8 head-chunks come first),
computes the token-importance mask redundantly, runs its 8 heads of attention,
and produces a partial to_out product over its 1024-wide d-slice for all 2048
output channels. The host sums the two partials per batch and adds bo'
(bo with the V-bias term folded in).

Key structure vs the previous version:
- x arrives as bf16 and is transposed into SBUF chunk-wise by the DMA XBAR
  (dma_start_transpose), removing all PE transposes and PSUM copies for x.
- V is projected directly into its PV-ready [token, channel] layout by using
  the x chunk as the stationary operand (out = xT_chunk.T @ WvT), removing
  the separate V transpose pass. The V bias is exactly handled outside the
  kernel: a = u/dn + bv*(S_pm/dn) with S_pm ~= dn + (sum(mask)-1024), so the
  per-head bias folds into an Act bias (k*bv) plus a host-side bo term.
- The softmax denominator is an M=1 ones-matmul accumulated over all 8
  j-tiles into a [1, N] PSUM row (ones = 1/32 so the normalized output is
  pre-scaled by 32 for fp8).
- to_out runs in fp8 DoubleRow (K=256/instr at 0.5 cycles/row) with an
  error-compensated hi+lo split of both Wo (host side, x64) and the
  attention output (device side, x32): w*a ~= w_hi*a_hi + w_hi*a_lo +
  w_lo*a_hi, 12 DR matmuls per (oc, half) instead of 16 bf16 matmuls.
"""

import sys

sys.path.insert(0, "/opt/trn_rl_repo")

import numpy as np
import ml_dtypes

import concourse.mybir as mybir
import concourse.tile as tile
from concourse import bacc, bass_utils
from concourse.tile import add_dep_helper

B = 4
N = 1024
C = 128
D = 2048
NCHUNK = 16  # d-chunks of 128 (= patch positions = heads)
HPC = 8  # heads per core
MASK_NUM = 25
SCALE = 64.0 ** -0.5  # 0.125

F32 = mybir.dt.float32
BF16 = mybir.dt.bfloat16
FP8 = mybir.dt.float8e4
U32 = mybir.dt.uint32
Exp = mybir.ActivationFunctionType.Exp
Ident = mybir.ActivationFunctionType.Identity
Ln = mybir.ActivationFunctionType.Ln
DR = mybir.MatmulPerfMode.DoubleRow
NEG_BIG = -1e30

WO_SCALE = 64.0  # host-side Wo prescale before fp8 split
A_SCALE = 32.0   # device-side attention-out prescale (via ones = 1/32)


def _body(tc, xc, wq_d, bq_d, wtc_d, wo_hi_d, wo_lo_d, outT_d):
    nc = tc.nc
    dscr = nc.dram_tensor("dscr", (HPC, N), F32, kind="Internal").ap()

    with (
        tc.tile_pool(name="consts", bufs=1) as consts,
        tc.tile_pool(name="persist", bufs=1) as persist,
    ):
        # ---- constants ----
        ones32 = consts.tile([128, 1], BF16)
        nc.vector.memset(ones32, 1.0 / A_SCALE)
        one_f32 = consts.tile([1, 1], F32)
        nc.vector.memset(one_f32, 1.0)
        ones128_f32 = consts.tile([1, 128], F32)
        nc.vector.memset(ones128_f32, 1.0)
        ones128_bf = consts.tile([1, 128], BF16)
        nc.vector.memset(ones128_bf, 1.0)

        # ---- persistent activations ----
        qT = persist.tile([128, HPC, N], BF16)      # [c', h, n] 2 MB
        kT = persist.tile([128, HPC, N], BF16)      # 2 MB
        vnat = persist.tile([128, HPC, 8, C], BF16)  # [j, h, jt, c] 2 MB
        outT_hi = persist.tile([128, HPC, N], FP8)  # 32*(a - bv) hi split
        outT_lo = persist.tile([128, HPC, N], FP8)
        wo_hi = persist.tile([128, HPC, D], FP8)    # [d, h-chunk, o] 2 MB
        wo_lo = persist.tile([128, HPC, D], FP8)
        mask_col = persist.tile([128, 8], F32)
        scale_col = persist.tile([128, 8], F32)
        lnm_col = persist.tile([128, 8], F32)       # ln(mask) exp bias
        kbv = persist.tile([128, 1], F32)           # (sum(m)-1024) * bv
        ksc32 = persist.tile([1, 1], F32)           # (sum(m)-1024)/32

        # ============ phase 1: x transpose-in, logits, mask, QKV ===========
        with (
            tc.tile_pool(name="ph1big", bufs=1) as ph1big,
            tc.tile_pool(name="mrows", bufs=1) as mrows,
            tc.tile_pool(name="mm_psum", bufs=2, space="PSUM") as mm_psum,
            tc.tile_pool(name="v_psum", bufs=2, space="PSUM") as v_psum,
        ):
            # packed weight loads first (tiny; scalar queue)
            wqkv_sb = consts.tile([C, 3 * C], BF16)
            nc.scalar.dma_start(out=wqkv_sb, in_=wq_d)
            wq_sb = wqkv_sb[:, 0:C]
            wk_sb = wqkv_sb[:, C : 2 * C]
            wv_sb = wqkv_sb[:, 2 * C : 3 * C]
            bqkv_sb = consts.tile([C, 3], F32)
            nc.scalar.dma_start(out=bqkv_sb, in_=bq_d)
            bq_sb = bqkv_sb[:, 0:1]
            bk_sb = bqkv_sb[:, 1:2]
            bv_sb = bqkv_sb[:, 2:3]
            wtc_sb = consts.tile([C, 1], BF16)
            nc.scalar.dma_start(out=wtc_sb, in_=wtc_d)

            # x transposed in by the DMA XBAR in 8 strided sweeps (pipelined
            # so logits can chase them). xc rows viewed as [(n k), c] with
            # row-stride 256B give layout xT[c, n, k] (k fastest on free).
            NP = 8
            PR = N // NP  # 128 tokens per piece
            xT = ph1big.tile([128, N, NCHUNK], BF16)  # [c, n, k] 4 MB
            xc_rows = xc.rearrange("n (k c) -> (n k) c", c=128)
            tp_insts = []
            for p in range(NP):
                hr = slice(p * PR * NCHUNK, (p + 1) * PR * NCHUNK)
                tp_insts.append(nc.sync.dma_start_transpose(
                    out=xT[:, p * PR : (p + 1) * PR, :],
                    in_=xc_rows[hr, :],
                ))

            # Wo hi/lo splits: one big DMA each, after the x transposes.
            for wo_sb, wo_src, dep in (
                (wo_hi, wo_hi_d, tp_insts[-2]),
                (wo_lo, wo_lo_d, tp_insts[-1]),
            ):
                wi = nc.gpsimd.dma_start(
                    out=wo_sb,
                    in_=wo_src.rearrange("(h p) o -> p h o", p=128),
                )
                add_dep_helper(wi.ins, dep.ins, sync=True, reason="wo after x")

            with tc.tile_pool(name="lg_psum", bufs=1, space="PSUM") as lg_psum:
                # logits[n] = sum_k xT[:, n, k] . wtc   (wtc = (Wl@Wq)/16),
                # one accumulation group per transpose piece so PE starts as
                # soon as the first piece lands.
                lg = lg_psum.tile([1, N], F32)
                negrow = mrows.tile([1, N], F32)
                for p in range(NP):
                    for k in range(NCHUNK):
                        nc.tensor.matmul(
                            lg[:, p * PR : (p + 1) * PR],
                            wtc_sb,
                            xT[:, p * PR : (p + 1) * PR, k],
                            start=(k == 0),
                            stop=(k == NCHUNK - 1),
                        )
                    # negate per piece: runs on DVE while later pieces land
                    nc.vector.tensor_scalar_mul(
                        negrow[:, p * PR : (p + 1) * PR],
                        lg[:, p * PR : (p + 1) * PR], -1.0,
                    )

                # ---- mask: softmax over tokens, snap all but 25 smallest
                # to 1. DVE runs the serial top-k; Act computes the softmax
                # normalization in parallel.
                m8 = mrows.tile([1, 8], F32)
                for _ in range(3):
                    nc.vector.max(out=m8, in_=negrow)
                    nc.vector.match_replace(
                        out=negrow, in_to_replace=m8, in_values=negrow,
                        imm_value=NEG_BIG,
                    )
                nc.vector.max(out=m8, in_=negrow)  # 25th largest of -L
                thrneg = mrows.tile([1, 1], F32)
                nc.vector.tensor_scalar_mul(thrneg, m8[:, 0:1], -1.0)
                urow = mrows.tile([1, N], F32)
                ssum = mrows.tile([1, 1], F32)
                nc.scalar.activation(
                    out=urow, in_=lg, func=Exp, accum_out=ssum
                )
                srecip = mrows.tile([1, 1], F32)
                nc.vector.reciprocal(srecip, ssum)
                smrow = mrows.tile([1, N], F32)
                nc.scalar.activation(
                    out=smrow, in_=urow, func=Ident, scale=srecip
                )
                sel = mrows.tile([1, N], U32)
                nc.vector.tensor_scalar(
                    sel, lg, thrneg, None, op0=mybir.AluOpType.is_gt
                )
                onesrow = mrows.tile([1, N], F32)
                nc.vector.memset(onesrow, 1.0)
                nc.vector.copy_predicated(smrow, sel, onesrow)
                # k = sum(mask) - 1024 (~ -25 + tiny): V-bias fold + dn fixup
                msum = mrows.tile([1, 1], F32)
                mdummy = mrows.tile([1, N], F32)
                nc.vector.tensor_scalar(
                    mdummy, smrow, 1.0, 0.0, op0=mybir.AluOpType.mult,
                    op1=mybir.AluOpType.add, accum_out=msum,
                )
                ksc = mrows.tile([1, 1], F32)
                nc.vector.tensor_scalar_add(ksc, msum, -float(N))
                nc.vector.tensor_scalar_mul(ksc32, ksc, 1.0 / A_SCALE)

            # ---- Q/K/V projections, interleaved per head -----------------
            # Q/K produce transposed layouts [c', h, n]; V goes directly to
            # its PV-ready [token, channel] layout (x chunk as stationary).
            # V bias is handled via kbv + host bo fold; the V mask lives in
            # the exp bias (lnm_col), so V copies have no mask dependency.
            # Act paces the Q/K bias-moves; V copies go to DVE (idle once
            # the mask chain drains) except the last heads on Act.
            for h in range(HPC):
                for jtg in range(2):
                    vp = v_psum.tile([128, 4, C], F32)
                    for dj in range(4):
                        jt = jtg * 4 + dj
                        nc.tensor.matmul(
                            vp[:, dj, :],
                            xT[:, jt * 128 : (jt + 1) * 128, h],
                            wv_sb,
                            start=True,
                            stop=True,
                        )
                    dst = vnat[:, h, jtg * 4 : (jtg + 1) * 4, :]
                    if h < 6:
                        nc.vector.tensor_copy(dst, vp)
                    else:
                        nc.scalar.activation(out=dst, in_=vp, func=Ident)
                for w_sb, b_sb, dstT in ((wq_sb, bq_sb, qT), (wk_sb, bk_sb, kT)):
                    pp = mm_psum.tile([128, N], F32)
                    for half in range(2):
                        nc.tensor.matmul(
                            pp[:, half * 512 : (half + 1) * 512],
                            w_sb,
                            xT[:, half * 512 : (half + 1) * 512, h],
                            start=True,
                            stop=True,
                        )
                    nc.scalar.activation(
                        out=dstT[:, h, :], in_=pp, func=Ident, bias=b_sb
                    )

            # ---- mask row -> [128, 8] columns via tiny PE transposes ------
            # (plus a [128,1] broadcast of k), replacing two DRAM bounces.
            with tc.tile_pool(name="mc_psum", bufs=1, space="PSUM") as mc_psum:
                mcol_ps = mc_psum.tile([128, 9], F32)
                for t in range(8):
                    nc.tensor.transpose(
                        mcol_ps[:, t : t + 1],
                        smrow[:, t * 128 : (t + 1) * 128],
                        one_f32,
                    )
                nc.tensor.matmul(
                    mcol_ps[:, 8:9], ones128_f32, ksc, start=True, stop=True
                )
                nc.vector.tensor_copy(mask_col, mcol_ps[:, 0:8])
                nc.vector.tensor_scalar_mul(scale_col, mask_col, SCALE)
                nc.scalar.activation(out=lnm_col, in_=mask_col, func=Ln)
                nc.vector.tensor_mul(kbv, mcol_ps[:, 8:9], bv_sb)

        # ================= phase 2: attention ==============================
        # Pool open order places dn/ot on the earliest-freed phase-1 banks.
        with (
            tc.tile_pool(name="pexp", bufs=2) as pexp_pool,
            tc.tile_pool(name="dvp", bufs=2) as dvp,
            tc.tile_pool(name="dn_psum", bufs=1, space="PSUM") as dn_psum,
            tc.tile_pool(name="ot_psum", bufs=1, space="PSUM") as ot_psum,
            tc.tile_pool(name="st_psum", bufs=2, space="PSUM") as st_psum,
        ):
            for h in range(HPC):
                ot = ot_psum.tile([128, N], F32, tag="ot")
                dn = dn_psum.tile([1, N], F32, tag="dn")
                pexp = pexp_pool.tile([128, 8, N], BF16, tag="pexp")

                def emit_pvdn(jt, ot=ot, dn=dn, pexp=pexp, h=h):
                    for half in range(2):
                        sl = slice(half * 512, (half + 1) * 512)
                        nc.tensor.matmul(
                            ot[:, sl],
                            vnat[:, h, jt, :],
                            pexp[:, jt, sl],
                            start=(jt == 0),
                            stop=(jt == 7),
                        )
                        nc.tensor.matmul(
                            dn[:, sl],
                            ones32,
                            pexp[:, jt, sl],
                            start=(jt == 0),
                            stop=(jt == 7),
                        )

                pending = None
                for jt in range(8):
                    st = st_psum.tile([128, N], F32, tag="st")
                    for half in range(2):
                        nc.tensor.matmul(
                            st[:, half * 512 : (half + 1) * 512],
                            kT[:, h, jt * 128 : (jt + 1) * 128],
                            qT[:, h, half * 512 : (half + 1) * 512],
                            start=True,
                            stop=True,
                        )
                    nc.scalar.activation(
                        out=pexp[:, jt, :], in_=st, func=Exp,
                        scale=scale_col[:, jt : jt + 1],
                        bias=lnm_col[:, jt : jt + 1],
                    )
                    if pending is not None:
                        emit_pvdn(pending)
                    pending = jt
                emit_pvdn(pending)

                # drain: otsb = u + k*bv (DVE per-partition add, frees ot);
                # dn' holds sum_j m_j p_j / 32, true dn/32 = dn' - k/32 (the
                # 25 masked tokens have p ~= 1); tmp = otsb * (32/dn);
                # hi/lo fp8 split of tmp.
                # otsb on Act: in phase 2 Act only runs exps, so this lands
                # right after exp7 and frees the (single-buffered) ot slot
                # for the next head's PV. On DVE it would queue behind the
                # previous head's rb-bounce-blocked tmp/hi/lo ops.
                otsb = dvp.tile([128, N], BF16, tag="otsb")
                nc.scalar.activation(out=otsb, in_=ot, func=Ident, bias=kbv)
                dnadj = dvp.tile([1, N], F32, tag="dnadj")
                nc.vector.tensor_scalar(
                    dnadj, dn, ksc32, None, op0=mybir.AluOpType.subtract
                )
                if h < HPC - 1:
                    rrow = dvp.tile([1, N], F32, tag="rrow")
                    nc.vector.reciprocal(rrow, dnadj)
                    w_i = nc.sync.dma_start(out=dscr[h, :], in_=rrow)
                    rb = dvp.tile([128, N], F32, tag="rb")
                    r_i = nc.sync.dma_start(
                        out=rb, in_=dscr[h, :].partition_broadcast(128)
                    )
                    add_dep_helper(r_i.ins, w_i.ins, sync=True,
                                   reason="recip RAW")
                else:
                    # last head gates to_out: broadcast 1/dn across partitions
                    # with a K=1 matmul into the freed ot slot instead of the
                    # (slower) DRAM round-trip.
                    rrow_bf = dvp.tile([1, N], BF16, tag="rrowbf")
                    with nc.allow_low_precision(
                        reason="1/dn broadcast row; 0.4% relative is fine"
                    ):
                        nc.vector.reciprocal(rrow_bf, dnadj)
                    rb = ot_psum.tile([128, N], F32, tag="ot")
                    for half in range(2):
                        sl = slice(half * 512, (half + 1) * 512)
                        nc.tensor.matmul(
                            rb[:, sl], ones128_bf, rrow_bf[:, sl],
                            start=True, stop=True,
                        )
                tmp = dvp.tile([128, N], BF16, tag="tmp")
                nc.vector.tensor_mul(tmp, otsb, rb)
                nc.vector.tensor_copy(outT_hi[:, h, :], tmp)
                nc.vector.tensor_sub(
                    outT_lo[:, h, :], tmp, outT_hi[:, h, :]
                )

            # ============= phase 4: to_out partial (fp8 DoubleRow) =========
            # fo shares the st_psum slots (same shape) so Wo accumulation can
            # begin as soon as the last exp frees an ST slot.
            with tc.tile_pool(name="fout", bufs=3) as fout_pool:
                PRODUCTS = ((wo_hi, outT_hi), (wo_hi, outT_lo), (wo_lo, outT_hi))

                def finish_oc(oc, fo):
                    fout = fout_pool.tile([128, N], BF16)
                    eng = nc.sync if oc % 2 == 0 else nc.gpsimd
                    if oc < 15:
                        nc.vector.tensor_scalar_mul(
                            fout, fo, 1.0 / (WO_SCALE * A_SCALE)
                        )
                        eng.dma_start(
                            out=outT_d[oc * 128 : (oc + 1) * 128, :], in_=fout
                        )
                    else:
                        # final oc: drain per half so the tail DMA covers
                        # only 512 columns
                        for sh in range(2):
                            ssl = slice(sh * 512, (sh + 1) * 512)
                            nc.vector.tensor_scalar_mul(
                                fout[:, ssl], fo[:, ssl],
                                1.0 / (WO_SCALE * A_SCALE),
                            )
                            eng = nc.sync if sh == 0 else nc.gpsimd
                            eng.dma_start(
                                out=outT_d[oc * 128 : (oc + 1) * 128, ssl],
                                in_=fout[:, ssl],
                            )

                pending_oc = None
                for oc in range(16):
                    fo = st_psum.tile([128, N], F32, tag="st")
                    for half in range(2):
                        sl = slice(half * 512, (half + 1) * 512)
                        # t=3 (heads 6,7) last in every product so the first
                        # 9 instructions don't wait on head 7's drain chain
                        steps = [(w, a, t) for t in (0, 1, 2)
                                 for (w, a) in PRODUCTS]
                        steps += [(w, a, 3) for (w, a) in PRODUCTS]
                        for i, (wsp, asp, t) in enumerate(steps):
                            nc.tensor.matmul(
                                fo[:, sl],
                                wsp[:, 2 * t : 2 * t + 2,
                                    oc * 128 : (oc + 1) * 128],
                                asp[:, 2 * t : 2 * t + 2, sl],
                                start=(i == 0),
                                stop=(i == len(steps) - 1),
                                perf_mode=DR,
                            )
                    if pending_oc is not None:
                        finish_oc(*pending_oc)
                    pending_oc = (oc, fo)
                finish_oc(*pending_oc)


_CACHE = {}


def _get_module():
    if "nc" in _CACHE:
        return _CACHE["nc"]
    nc = bacc.Bacc("TRN2", target_bir_lowering=False, debug=False, num_devices=8)
    xc = nc.dram_tensor("xc", (N, D), BF16, kind="ExternalInput").ap()
    wq_d = nc.dram_tensor("wqkvT", (C, 3 * C), BF16, kind="ExternalInput").ap()
    bq_d = nc.dram_tensor("bqkv", (C, 3), F32, kind="ExternalInput").ap()
    wtc_d = nc.dram_tensor("wtc", (C, 1), BF16, kind="ExternalInput").ap()
    wo_hi_d = nc.dram_tensor("woHi", (HPC * C, D), FP8, kind="ExternalInput").ap()
    wo_lo_d = nc.dram_tensor("woLo", (HPC * C, D), FP8, kind="ExternalInput").ap()
    outT_d = nc.dram_tensor("outT", (D, N), BF16, kind="ExternalOutput").ap()

    with tile.TileContext(nc) as tc:
        _body(tc, xc, wq_d, bq_d, wtc_d, wo_hi_d, wo_lo_d, outT_d)
    nc.compile()
    _CACHE["nc"] = nc
    return nc


def make_in_maps(x, Wq, bq, Wk, bk, Wv, bv, Wl, bl, Wo, bo):
    bf16 = ml_dtypes.bfloat16
    fp8 = ml_dtypes.float8_e4m3
    x = np.ascontiguousarray(np.asarray(x, np.float32))
    Wq = np.asarray(Wq, np.float32)
    Wk = np.asarray(Wk, np.float32)
    Wv = np.asarray(Wv, np.float32)
    Wl = np.asarray(Wl, np.float32)
    Wo = np.asarray(Wo, np.float32)
    we = (Wl[0] @ Wq) / float(NCHUNK)  # (128,) logits weight per chunk
    common = {
        "wqkvT": np.ascontiguousarray(
            np.concatenate([Wq.T, Wk.T, Wv.T], axis=1)
        ).astype(bf16),
        "bqkv": np.ascontiguousarray(np.stack(
            [np.asarray(bq, np.float32), np.asarray(bk, np.float32),
             np.asarray(bv, np.float32)], axis=1
        )),
        "wtc": we.astype(bf16).reshape(C, 1),
    }
    woT = np.ascontiguousarray(Wo.T) * WO_SCALE  # (d, o), prescaled
    in_maps = []
    halves = {}
    for g in range(2):
        wh = woT[g * 1024 : (g + 1) * 1024, :]
        hi = wh.astype(fp8)
        lo = (wh - hi.astype(np.float32)).astype(fp8)
        halves[g] = (np.ascontiguousarray(hi), np.ascontiguousarray(lo))
    for core in range(8):
        b, g = divmod(core, 2)
        xb = x[b]
        xcore = xb if g == 0 else np.concatenate(
            [xb[:, 1024:], xb[:, :1024]], axis=1
        )
        in_maps.append({
            "xc": np.ascontiguousarray(xcore.astype(bf16)),
            "woHi": halves[g][0],
            "woLo": halves[g][1],
            **common,
        })
    return in_maps


def run_spmd(in_maps, trace=False, **kw):
    nc = _get_module()
    return bass_utils.run_bass_kernel_spmd(
        nc, in_maps, core_ids=list(range(8)), trace=trace, **kw
    )


def gather(results, Wo, bv, bo):
    Wo = np.asarray(Wo, np.float32)
    bv = np.asarray(bv, np.float32)
    bo = np.asarray(bo, np.float32)
    # a = a_tilde + bv per head-channel: fold bv through Wo into the bias.
    bo_eff = bo + np.tile(bv, NCHUNK) @ Wo.T
    out = np.empty((B, N, D), np.float32)
    for b in range(B):
        p0 = results[2 * b]["outT"].astype(np.float32).T
        p1 = results[2 * b + 1]["outT"].astype(np.float32).T
        out[b] = p0 + p1 + bo_eff
    return out


def kernel(x, Wq, bq, Wk, bk, Wv, bv, Wl, bl, Wo, bo, stage=None, **_unused):
    in_maps = make_in_maps(x, Wq, bq, Wk, bk, Wv, bv, Wl, bl, Wo, bo)
    try:
        res = run_spmd(in_maps)
    except Exception:
        # transient device/runtime hiccup: retry once after a short pause
        import time as _time

        _time.sleep(2.0)
        res = run_spmd(in_maps)
    return gather(res.results, Wo, bv, bo)


# revision 46
# speedup vs baseline: 1.0262x; 1.0079x over previous
"""Trainium2 Bass kernel for nn_Attention_54614804136573 (topk_masking).

Sharding: 8 cores = 4 batches x 2 head-groups (8 heads each). Each core gets
its batch's full x (columns rotated so its own 8 head-chunks come first),
computes the token-importance mask redundantly, runs its 8 heads of attention,
and produces a partial to_out product over its 1024-wide d-slice for all 2048
output channels. The host sums the two partials per batch and adds bo'
(bo with the V-bias term folded in).

Key structure vs the previous version:
- x arrives as bf16 and is transposed into SBUF chunk-wise by the DMA XBAR
  (dma_start_transpose), removing all PE transposes and PSUM copies for x.
- V is projected directly into its PV-ready [token, channel] layout by using
  the x chunk as the stationary operand (out = xT_chunk.T @ WvT), removing
  the separate V transpose pass. The V bias is exactly handled outside the
  kernel: a = u/dn + bv*(S_pm/dn) with S_pm ~= dn + (sum(mask)-1024), so the
  per-head bias folds into an Act bias (k*bv) plus a host-side bo term.
- The softmax denominator is an M=1 ones-matmul accumulated over all 8
  j-tiles into a [1, N] PSUM row (ones = 1/32 so the normalized output is
  pre-scaled by 32 for fp8).
- to_out runs in fp8 DoubleRow (K=256/instr at 0.5 cycles/row) with an
  error-compensated hi+lo split of both Wo (host side, x64) and the
  attention output (device side, x32): w*a ~= w_hi*a_hi + w_hi*a_lo +
  w_lo*a_hi, 12 DR matmuls per (oc, half) instead of 16 bf16 matmuls.
"""

import sys

sys.path.insert(0, "/opt/trn_rl_repo")

import numpy as np
import ml_dtypes

import concourse.mybir as mybir
import concourse.tile as tile
from concourse import bacc, bass_utils
from concourse.tile import add_dep_helper

B = 4
N = 1024
C = 128
D = 2048
NCHUNK = 16  # d-chunks of 128 (= patch positions = heads)
HPC = 8  # heads per core
MASK_NUM = 25
SCALE = 64.0 ** -0.5  # 0.125

F32 = mybir.dt.float32
BF16 = mybir.dt.bfloat16
FP8 = mybir.dt.float8e4
U32 = mybir.dt.uint32
Exp = mybir.ActivationFunctionType.Exp
Ident = mybir.ActivationFunctionType.Identity
Ln = mybir.ActivationFunctionType.Ln
DR = mybir.MatmulPerfMode.DoubleRow
NEG_BIG = -1e30

WO_SCALE = 64.0  # host-side Wo prescale before fp8 split
A_SCALE = 32.0   # device-side attention-out prescale (via ones = 1/32)


def _body(tc, xc, wq_d, bq_d, wtc_d, wo_hi_d, wo_lo_d, outT_d):
    nc = tc.nc
    dscr = nc.dram_tensor("dscr", (HPC, N), F32, kind="Internal").ap()

    with (
        tc.tile_pool(name="consts", bufs=1) as consts,
        tc.tile_pool(name="persist", bufs=1) as persist,
    ):
        # ---- constants ----
        ones32 = consts.tile([128, 1], BF16)
        nc.vector.memset(ones32, 1.0 / A_SCALE)
        one_f32 = consts.tile([1, 1], F32)
        nc.vector.memset(one_f32, 1.0)
        ones128_f32 = consts.tile([1, 128], F32)
        nc.vector.memset(ones128_f32, 1.0)
        ones128_bf = consts.tile([1, 128], BF16)
        nc.vector.memset(ones128_bf, 1.0)

        # ---- persistent activations ----
        qT = persist.tile([128, HPC, N], BF16)      # [c', h, n] 2 MB
        kT = persist.tile([128, HPC, N], BF16)      # 2 MB
        vnat = persist.tile([128, HPC, 8, C], BF16)  # [j, h, jt, c] 2 MB
        outT_hi = persist.tile([128, HPC, N], FP8)  # 32*(a - bv) hi split
        outT_lo = persist.tile([128, HPC, N], FP8)
        wo_hi = persist.tile([128, HPC, D], FP8)    # [d, h-chunk, o] 2 MB
        wo_lo = persist.tile([128, HPC, D], FP8)
        mask_col = persist.tile([128, 8], F32)
        scale_col = persist.tile([128, 8], F32)
        lnm_col = persist.tile([128, 8], F32)       # ln(mask) exp bias
        kbv = persist.tile([128, 1], F32)           # (sum(m)-1024) * bv
        ksc32 = persist.tile([1, 1], F32)           # (sum(m)-1024)/32

        # ============ phase 1: x transpose-in, logits, mask, QKV ===========
        with (
            tc.tile_pool(name="ph1big", bufs=1) as ph1big,
            tc.tile_pool(name="mrows", bufs=1) as mrows,
            tc.tile_pool(name="mm_psum", bufs=2, space="PSUM") as mm_psum,
            tc.tile_pool(name="v_psum", bufs=2, space="PSUM") as v_psum,
        ):
            # packed weight loads first (tiny; scalar queue)
            wqkv_sb = consts.tile([C, 3 * C], BF16)
            nc.scalar.dma_start(out=wqkv_sb, in_=wq_d)
            wq_sb = wqkv_sb[:, 0:C]
            wk_sb = wqkv_sb[:, C : 2 * C]
            wv_sb = wqkv_sb[:, 2 * C : 3 * C]
            bqkv_sb = consts.tile([C, 3], F32)
            nc.scalar.dma_start(out=bqkv_sb, in_=bq_d)
            bq_sb = bqkv_sb[:, 0:1]
            bk_sb = bqkv_sb[:, 1:2]
            bv_sb = bqkv_sb[:, 2:3]
            wtc_sb = consts.tile([C, 1], BF16)
            nc.scalar.dma_start(out=wtc_sb, in_=wtc_d)

            # x transposed in by the DMA XBAR in 8 strided sweeps (pipelined
            # so logits can chase them). xc rows viewed as [(n k), c] with
            # row-stride 256B give layout xT[c, n, k] (k fastest on free).
            NP = 8
            PR = N // NP  # 128 tokens per piece
            xT = ph1big.tile([128, N, NCHUNK], BF16)  # [c, n, k] 4 MB
            xc_rows = xc.rearrange("n (k c) -> (n k) c", c=128)
            tp_insts = []
            for p in range(NP):
                hr = slice(p * PR * NCHUNK, (p + 1) * PR * NCHUNK)
                tp_insts.append(nc.sync.dma_start_transpose(
                    out=xT[:, p * PR : (p + 1) * PR, :],
                    in_=xc_rows[hr, :],
                ))

            # Wo hi/lo splits: one big DMA each, after the x transposes.
            for wo_sb, wo_src, dep in (
                (wo_hi, wo_hi_d, tp_insts[-2]),
                (wo_lo, wo_lo_d, tp_insts[-1]),
            ):
                wi = nc.gpsimd.dma_start(
                    out=wo_sb,
                    in_=wo_src.rearrange("(h p) o -> p h o", p=128),
                )
                add_dep_helper(wi.ins, dep.ins, sync=True, reason="wo after x")

            with tc.tile_pool(name="lg_psum", bufs=1, space="PSUM") as lg_psum:
                # logits[n] = sum_k xT[:, n, k] . wtc   (wtc = (Wl@Wq)/16),
                # one accumulation group per transpose piece so PE starts as
                # soon as the first piece lands.
                lg = lg_psum.tile([1, N], F32)
                negrow = mrows.tile([1, N], F32)
                for p in range(NP):
                    for k in range(NCHUNK):
                        nc.tensor.matmul(
                            lg[:, p * PR : (p + 1) * PR],
                            wtc_sb,
                            xT[:, p * PR : (p + 1) * PR, k],
                            start=(k == 0),
                            stop=(k == NCHUNK - 1),
                        )
                    # negate per piece: runs on DVE while later pieces land
                    nc.vector.tensor_scalar_mul(
                        negrow[:, p * PR : (p + 1) * PR],
                        lg[:, p * PR : (p + 1) * PR], -1.0,
                    )

                # ---- mask: softmax over tokens, snap all but 25 smallest
                # to 1. DVE runs the serial top-k; Act computes the softmax
                # normalization in parallel.
                m8 = mrows.tile([1, 8], F32)
                for _ in range(3):
                    nc.vector.max(out=m8, in_=negrow)
                    nc.vector.match_replace(
                        out=negrow, in_to_replace=m8, in_values=negrow,
                        imm_value=NEG_BIG,
                    )
                nc.vector.max(out=m8, in_=negrow)  # 25th largest of -L
                thrneg = mrows.tile([1, 1], F32)
                nc.vector.tensor_scalar_mul(thrneg, m8[:, 0:1], -1.0)
                urow = mrows.tile([1, N], F32)
                ssum = mrows.tile([1, 1], F32)
                nc.scalar.activation(
                    out=urow, in_=lg, func=Exp, accum_out=ssum
                )
                srecip = mrows.tile([1, 1], F32)
                nc.vector.reciprocal(srecip, ssum)
                smrow = mrows.tile([1, N], F32)
                nc.scalar.activation(
                    out=smrow, in_=urow, func=Ident, scale=srecip
                )
                sel = mrows.tile([1, N], U32)
                nc.vector.tensor_scalar(
                    sel, lg, thrneg, None, op0=mybir.AluOpType.is_gt
                )
                onesrow = mrows.tile([1, N], F32)
                nc.vector.memset(onesrow, 1.0)
                nc.vector.copy_predicated(smrow, sel, onesrow)
                # k = sum(mask) - 1024 (~ -25 + tiny): V-bias fold + dn fixup
                msum = mrows.tile([1, 1], F32)
                mdummy = mrows.tile([1, N], F32)
                nc.vector.tensor_scalar(
                    mdummy, smrow, 1.0, 0.0, op0=mybir.AluOpType.mult,
                    op1=mybir.AluOpType.add, accum_out=msum,
                )
                ksc = mrows.tile([1, 1], F32)
                nc.vector.tensor_scalar_add(ksc, msum, -float(N))
                nc.vector.tensor_scalar_mul(ksc32, ksc, 1.0 / A_SCALE)

            # ---- Q/K/V projections, interleaved per head -----------------
            # Q/K produce transposed layouts [c', h, n]; V goes directly to
            # its PV-ready [token, channel] layout (x chunk as stationary).
            # V bias is handled via kbv + host bo fold; the V mask lives in
            # the exp bias (lnm_col), so V copies have no mask dependency.
            # Act paces the Q/K bias-moves; V copies go to DVE (idle once
            # the mask chain drains) except the last heads on Act.
            for h in range(HPC):
                for jtg in range(2):
                    vp = v_psum.tile([128, 4, C], F32)
                    for dj in range(4):
                        jt = jtg * 4 + dj
                        nc.tensor.matmul(
                            vp[:, dj, :],
                            xT[:, jt * 128 : (jt + 1) * 128, h],
                            wv_sb,
                            start=True,
                            stop=True,
                        )
                    dst = vnat[:, h, jtg * 4 : (jtg + 1) * 4, :]
                    if h < 6:
                        nc.vector.tensor_copy(dst, vp)
                    else:
                        nc.scalar.activation(out=dst, in_=vp, func=Ident)
                for w_sb, b_sb, dstT in ((wq_sb, bq_sb, qT), (wk_sb, bk_sb, kT)):
                    pp = mm_psum.tile([128, N], F32)
                    for half in range(2):
                        nc.tensor.matmul(
                            pp[:, half * 512 : (half + 1) * 512],
                            w_sb,
                            xT[:, half * 512 : (half + 1) * 512, h],
                            start=True,
                            stop=True,
                        )
                    nc.scalar.activation(
                        out=dstT[:, h, :], in_=pp, func=Ident, bias=b_sb
                    )

            # ---- mask row -> [128, 8] columns via tiny PE transposes ------
            # (plus a [128,1] broadcast of k), replacing two DRAM bounces.
            with tc.tile_pool(name="mc_psum", bufs=1, space="PSUM") as mc_psum:
                mcol_ps = mc_psum.tile([128, 9], F32)
                for t in range(8):
                    nc.tensor.transpose(
                        mcol_ps[:, t : t + 1],
                        smrow[:, t * 128 : (t + 1) * 128],
                        one_f32,
                    )
                nc.tensor.matmul(
                    mcol_ps[:, 8:9], ones128_f32, ksc, start=True, stop=True
                )
                nc.vector.tensor_copy(mask_col, mcol_ps[:, 0:8])
                nc.vector.tensor_scalar_mul(scale_col, mask_col, SCALE)
                nc.scalar.activation(out=lnm_col, in_=mask_col, func=Ln)
                nc.vector.tensor_mul(kbv, mcol_ps[:, 8:9], bv_sb)

        # ================= phase 2: attention ==============================
        # Pool open order places dn/ot on the earliest-freed phase-1 banks.
        with (
            tc.tile_pool(name="pexp", bufs=2) as pexp_pool,
            tc.tile_pool(name="dvp", bufs=2) as dvp,
            tc.tile_pool(name="dn_psum", bufs=1, space="PSUM") as dn_psum,
            tc.tile_pool(name="ot_psum", bufs=1, space="PSUM") as ot_psum,
            tc.tile_pool(name="st_psum", bufs=2, space="PSUM") as st_psum,
        ):
            for h in range(HPC):
                ot = ot_psum.tile([128, N], F32, tag="ot")
                dn = dn_psum.tile([1, N], F32, tag="dn")
                pexp = pexp_pool.tile([128, 8, N], BF16, tag="pexp")

                def emit_pvdn(jt, ot=ot, dn=dn, pexp=pexp, h=h):
                    for half in range(2):
                        sl = slice(half * 512, (half + 1) * 512)
                        nc.tensor.matmul(
                            ot[:, sl],
                            vnat[:, h, jt, :],
                            pexp[:, jt, sl],
                            start=(jt == 0),
                            stop=(jt == 7),
                        )
                        nc.tensor.matmul(
                            dn[:, sl],
                            ones32,
                            pexp[:, jt, sl],
                            start=(jt == 0),
                            stop=(jt == 7),
                        )

                pending = None
                for jt in range(8):
                    st = st_psum.tile([128, N], F32, tag="st")
                    for half in range(2):
                        nc.tensor.matmul(
                            st[:, half * 512 : (half + 1) * 512],
                            kT[:, h, jt * 128 : (jt + 1) * 128],
                            qT[:, h, half * 512 : (half + 1) * 512],
                            start=True,
                            stop=True,
                        )
                    nc.scalar.activation(
                        out=pexp[:, jt, :], in_=st, func=Exp,
                        scale=scale_col[:, jt : jt + 1],
                        bias=lnm_col[:, jt : jt + 1],
                    )
                    if pending is not None:
                        emit_pvdn(pending)
                    pending = jt
                emit_pvdn(pending)

                # drain: otsb = u + k*bv (DVE per-partition add, frees ot);
                # dn' holds sum_j m_j p_j / 32, true dn/32 = dn' - k/32 (the
                # 25 masked tokens have p ~= 1); tmp = otsb * (32/dn);
                # hi/lo fp8 split of tmp.
                otsb = dvp.tile([128, N], BF16, tag="otsb")
                if h < HPC - 1:
                    nc.vector.tensor_scalar(
                        otsb, ot, kbv, None, op0=mybir.AluOpType.add
                    )
                else:
                    # last head: Act is idle here; keep DVE free for the
                    # to_out-gating dnadj/recip/tmp/hi/lo chain
                    nc.scalar.activation(
                        out=otsb, in_=ot, func=Ident, bias=kbv
                    )
                dnadj = dvp.tile([1, N], F32, tag="dnadj")
                nc.vector.tensor_scalar(
                    dnadj, dn, ksc32, None, op0=mybir.AluOpType.subtract
                )
                if h < HPC - 1:
                    rrow = dvp.tile([1, N], F32, tag="rrow")
                    nc.vector.reciprocal(rrow, dnadj)
                    w_i = nc.sync.dma_start(out=dscr[h, :], in_=rrow)
                    rb = dvp.tile([128, N], F32, tag="rb")
                    r_i = nc.sync.dma_start(
                        out=rb, in_=dscr[h, :].partition_broadcast(128)
                    )
                    add_dep_helper(r_i.ins, w_i.ins, sync=True,
                                   reason="recip RAW")
                else:
                    # last head gates to_out: broadcast 1/dn across partitions
                    # with a K=1 matmul into the freed ot slot instead of the
                    # (slower) DRAM round-trip.
                    rrow_bf = dvp.tile([1, N], BF16, tag="rrowbf")
                    with nc.allow_low_precision(
                        reason="1/dn broadcast row; 0.4% relative is fine"
                    ):
                        nc.vector.reciprocal(rrow_bf, dnadj)
                    rb = ot_psum.tile([128, N], F32, tag="ot")
                    for half in range(2):
                        sl = slice(half * 512, (half + 1) * 512)
                        nc.tensor.matmul(
                            rb[:, sl], ones128_bf, rrow_bf[:, sl],
                            start=True, stop=True,
                        )
                tmp = dvp.tile([128, N], BF16, tag="tmp")
                nc.vector.tensor_mul(tmp, otsb, rb)
                nc.vector.tensor_copy(outT_hi[:, h, :], tmp)
                nc.vector.tensor_sub(
                    outT_lo[:, h, :], tmp, outT_hi[:, h, :]
                )

            # ============= phase 4: to_out partial (fp8 DoubleRow) =========
            # fo shares the st_psum slots (same shape) so Wo accumulation can
            # begin as soon as the last exp frees an ST slot.
            with tc.tile_pool(name="fout", bufs=3) as fout_pool:
                PRODUCTS = ((wo_hi, outT_hi), (wo_hi, outT_lo), (wo_lo, outT_hi))

                def finish_oc(oc, fo):
                    fout = fout_pool.tile([128, N], BF16)
                    eng = nc.sync if oc % 2 == 0 else nc.gpsimd
                    if oc < 15:
                        nc.vector.tensor_scalar_mul(
                            fout, fo, 1.0 / (WO_SCALE * A_SCALE)
                        )
                        eng.dma_start(
                            out=outT_d[oc * 128 : (oc + 1) * 128, :], in_=fout
                        )
                    else:
                        # final oc: drain per half so the tail DMA covers
                        # only 512 columns
                        for sh in range(2):
                            ssl = slice(sh * 512, (sh + 1) * 512)
                            nc.vector.tensor_scalar_mul(
                                fout[:, ssl], fo[:, ssl],
                                1.0 / (WO_SCALE * A_SCALE),
                            )
                            eng = nc.sync if sh == 0 else nc.gpsimd
                            eng.dma_start(
                                out=outT_d[oc * 128 : (oc + 1) * 128, ssl],
                                in_=fout[:, ssl],
                            )

                pending_oc = None
                for oc in range(16):
                    fo = st_psum.tile([128, N], F32, tag="st")
                    for half in range(2):
                        sl = slice(half * 512, (half + 1) * 512)
                        # t=3 (heads 6,7) last in every product so the first
                        # 9 instructions don't wait on head 7's drain chain
                        steps = [(w, a, t) for t in (0, 1, 2)
                                 for (w, a) in PRODUCTS]
                        steps += [(w, a, 3) for (w, a) in PRODUCTS]
                        for i, (wsp, asp, t) in enumerate(steps):
                            nc.tensor.matmul(
                                fo[:, sl],
                                wsp[:, 2 * t : 2 * t + 2,
                                    oc * 128 : (oc + 1) * 128],
                                asp[:, 2 * t : 2 * t + 2, sl],
                                start=(i == 0),
                                stop=(i == len(steps) - 1),
                                perf_mode=DR,
                            )
                    if pending_oc is not None:
                        finish_oc(*pending_oc)
                    pending_oc = (oc, fo)
                finish_oc(*pending_oc)


_CACHE = {}


def _get_module():
    if "nc" in _CACHE:
        return _CACHE["nc"]
    nc = bacc.Bacc("TRN2", target_bir_lowering=False, debug=False, num_devices=8)
    xc = nc.dram_tensor("xc", (N, D), BF16, kind="ExternalInput").ap()
    wq_d = nc.dram_tensor("wqkvT", (C, 3 * C), BF16, kind="ExternalInput").ap()
    bq_d = nc.dram_tensor("bqkv", (C, 3), F32, kind="ExternalInput").ap()
    wtc_d = nc.dram_tensor("wtc", (C, 1), BF16, kind="ExternalInput").ap()
    wo_hi_d = nc.dram_tensor("woHi", (HPC * C, D), FP8, kind="ExternalInput").ap()
    wo_lo_d = nc.dram_tensor("woLo", (HPC * C, D), FP8, kind="ExternalInput").ap()
    outT_d = nc.dram_tensor("outT", (D, N), BF16, kind="ExternalOutput").ap()

    with tile.TileContext(nc) as tc:
        _body(tc, xc, wq_d, bq_d, wtc_d, wo_hi_d, wo_lo_d, outT_d)
    nc.compile()
    _CACHE["nc"] = nc
    return nc


def make_in_maps(x, Wq, bq, Wk, bk, Wv, bv, Wl, bl, Wo, bo):
    bf16 = ml_dtypes.bfloat16
    fp8 = ml_dtypes.float8_e4m3
    x = np.ascontiguousarray(np.asarray(x, np.float32))
    Wq = np.asarray(Wq, np.float32)
    Wk = np.asarray(Wk, np.float32)
    Wv = np.asarray(Wv, np.float32)
    Wl = np.asarray(Wl, np.float32)
    Wo = np.asarray(Wo, np.float32)
    we = (Wl[0] @ Wq) / float(NCHUNK)  # (128,) logits weight per chunk
    common = {
        "wqkvT": np.ascontiguousarray(
            np.concatenate([Wq.T, Wk.T, Wv.T], axis=1)
        ).astype(bf16),
        "bqkv": np.ascontiguousarray(np.stack(
            [np.asarray(bq, np.float32), np.asarray(bk, np.float32),
             np.asarray(bv, np.float32)], axis=1
        )),
        "wtc": we.astype(bf16).reshape(C, 1),
    }
    woT = np.ascontiguousarray(Wo.T) * WO_SCALE  # (d, o), prescaled
    in_maps = []
    halves = {}
    for g in range(2):
        wh = woT[g * 1024 : (g + 1) * 1024, :]
        hi = wh.astype(fp8)
        lo = (wh - hi.astype(np.float32)).astype(fp8)
        halves[g] = (np.ascontiguousarray(hi), np.ascontiguousarray(lo))
    for core in range(8):
        b, g = divmod(core, 2)
        xb = x[b]
        xcore = xb if g == 0 else np.concatenate(
            [xb[:, 1024:], xb[:, :1024]], axis=1
        )
        in_maps.append({
            "xc": np.ascontiguousarray(xcore.astype(bf16)),
            "woHi": halves[g][0],
            "woLo": halves[g][1],
            **common,
        })
    return in_maps


def run_spmd(in_maps, trace=False, **kw):
    nc = _get_module()
    return bass_utils.run_bass_kernel_spmd(
        nc, in_maps, core_ids=list(range(8)), trace=trace, **kw
    )


def gather(results, Wo, bv, bo):
    Wo = np.asarray(Wo, np.float32)
    bv = np.asarray(bv, np.float32)
    bo = np.asarray(bo, np.float32)
    # a = a_tilde + bv per head-channel: fold bv through Wo into the bias.
    bo_eff = bo + np.tile(bv, NCHUNK) @ Wo.T
    out = np.empty((B, N, D), np.float32)
    for b in range(B):
        p0 = results[2 * b]["outT"].astype(np.float32).T
        p1 = results[2 * b + 1]["outT"].astype(np.float32).T
        out[b] = p0 + p1 + bo_eff
    return out


def kernel(x, Wq, bq, Wk, bk, Wv, bv, Wl, bl, Wo, bo, stage=None, **_unused):
    in_maps = make_in_maps(x, Wq, bq, Wk, bk, Wv, bv, Wl, bl, Wo, bo)
    try:
        res = run_spmd(in_maps)
    except Exception:
        # transient device/runtime hiccup: retry once after a short pause
        import time as _time

        _time.sleep(2.0)
        res = run_spmd(in_maps)
    return gather(res.results, Wo, bv, bo)


# revision 49
# speedup vs baseline: 1.0303x; 1.0040x over previous
"""Trainium2 Bass kernel for nn_Attention_54614804136573 (topk_masking).

Sharding: 8 cores = 4 batches x 2 head-groups (8 heads each). Each core gets
its batch's full x (columns rotated so its own 8 head-chunks come first),
computes the token-importance mask redundantly, runs its 8 heads of attention,
and produces a partial to_out product over its 1024-wide d-slice for all 2048
output channels. The host sums the two partials per batch and adds bo'
(bo with the V-bias term folded in).

Key structure vs the previous version:
- x arrives as bf16 and is transposed into SBUF chunk-wise by the DMA XBAR
  (dma_start_transpose), removing all PE transposes and PSUM copies for x.
- V is projected directly into its PV-ready [token, channel] layout by using
  the x chunk as the stationary operand (out = xT_chunk.T @ WvT), removing
  the separate V transpose pass. The V bias is exactly handled outside the
  kernel: a = u/dn + bv*(S_pm/dn) with S_pm ~= dn + (sum(mask)-1024), so the
  per-head bias folds into an Act bias (k*bv) plus a host-side bo term.
- The softmax denominator is an M=1 ones-matmul accumulated over all 8
  j-tiles into a [1, N] PSUM row (ones = 1/32 so the normalized output is
  pre-scaled by 32 for fp8).
- to_out runs in fp8 DoubleRow (K=256/instr at 0.5 cycles/row) with an
  error-compensated hi+lo split of both Wo (host side, x64) and the
  attention output (device side, x32): w*a ~= w_hi*a_hi + w_hi*a_lo +
  w_lo*a_hi, 12 DR matmuls per (oc, half) instead of 16 bf16 matmuls.
"""

import sys

sys.path.insert(0, "/opt/trn_rl_repo")

import numpy as np
import ml_dtypes

import concourse.mybir as mybir
import concourse.tile as tile
from concourse import bacc, bass_utils
from concourse.tile import add_dep_helper

B = 4
N = 1024
C = 128
D = 2048
NCHUNK = 16  # d-chunks of 128 (= patch positions = heads)
HPC = 8  # heads per core
MASK_NUM = 25
SCALE = 64.0 ** -0.5  # 0.125

F32 = mybir.dt.float32
BF16 = mybir.dt.bfloat16
FP8 = mybir.dt.float8e4
U32 = mybir.dt.uint32
Exp = mybir.ActivationFunctionType.Exp
Ident = mybir.ActivationFunctionType.Identity
Ln = mybir.ActivationFunctionType.Ln
DR = mybir.MatmulPerfMode.DoubleRow
NEG_BIG = -1e30

WO_SCALE = 64.0  # host-side Wo prescale before fp8 split
A_SCALE = 32.0   # device-side attention-out prescale (via ones = 1/32)


def _body(tc, xc, wq_d, bq_d, wtc_d, wo_hi_d, wo_lo_d, outT_d):
    nc = tc.nc
    dscr = nc.dram_tensor("dscr", (HPC, N), F32, kind="Internal").ap()

    with (
        tc.tile_pool(name="consts", bufs=1) as consts,
        tc.tile_pool(name="persist", bufs=1) as persist,
    ):
        # ---- constants ----
        ones32 = consts.tile([128, 1], BF16)
        nc.vector.memset(ones32, 1.0 / A_SCALE)
        one_f32 = consts.tile([1, 1], F32)
        nc.vector.memset(one_f32, 1.0)
        ones128_f32 = consts.tile([1, 128], F32)
        nc.vector.memset(ones128_f32, 1.0)
        ones128_bf = consts.tile([1, 128], BF16)
        nc.vector.memset(ones128_bf, 1.0)

        # ---- persistent activations ----
        qT = persist.tile([128, HPC, N], BF16)      # [c', h, n] 2 MB
        kT = persist.tile([128, HPC, N], BF16)      # 2 MB
        vnat = persist.tile([128, HPC, 8, C], BF16)  # [j, h, jt, c] 2 MB
        outT_hi = persist.tile([128, HPC, N], FP8)  # 32*(a - bv) hi split
        outT_lo = persist.tile([128, HPC, N], FP8)
        wo_hi = persist.tile([128, HPC, D], FP8)    # [d, h-chunk, o] 2 MB
        wo_lo = persist.tile([128, HPC, D], FP8)
        mask_col = persist.tile([128, 8], F32)
        scale_col = persist.tile([128, 8], F32)
        lnm_col = persist.tile([128, 8], F32)       # ln(mask) exp bias
        kbv = persist.tile([128, 1], F32)           # (sum(m)-1024) * bv
        ksc32 = persist.tile([1, 1], F32)           # (sum(m)-1024)/32

        # ============ phase 1: x transpose-in, logits, mask, QKV ===========
        with (
            tc.tile_pool(name="ph1big", bufs=1) as ph1big,
            tc.tile_pool(name="mrows", bufs=1) as mrows,
            tc.tile_pool(name="mm_psum", bufs=2, space="PSUM") as mm_psum,
            tc.tile_pool(name="v_psum", bufs=2, space="PSUM") as v_psum,
        ):
            # packed weight loads first (tiny; scalar queue)
            wqkv_sb = consts.tile([C, 3 * C], BF16)
            nc.scalar.dma_start(out=wqkv_sb, in_=wq_d)
            wq_sb = wqkv_sb[:, 0:C]
            wk_sb = wqkv_sb[:, C : 2 * C]
            wv_sb = wqkv_sb[:, 2 * C : 3 * C]
            bqkv_sb = consts.tile([C, 3], F32)
            nc.scalar.dma_start(out=bqkv_sb, in_=bq_d)
            bq_sb = bqkv_sb[:, 0:1]
            bk_sb = bqkv_sb[:, 1:2]
            bv_sb = bqkv_sb[:, 2:3]
            wtc_sb = consts.tile([C, 1], BF16)
            nc.scalar.dma_start(out=wtc_sb, in_=wtc_d)

            # x transposed in by the DMA XBAR in 8 strided sweeps (pipelined
            # so logits can chase them). xc rows viewed as [(n k), c] with
            # row-stride 256B give layout xT[c, n, k] (k fastest on free).
            NP = 8
            PR = N // NP  # 128 tokens per piece
            xT = ph1big.tile([128, N, NCHUNK], BF16)  # [c, n, k] 4 MB
            xc_rows = xc.rearrange("n (k c) -> (n k) c", c=128)
            tp_insts = []
            for p in range(NP):
                hr = slice(p * PR * NCHUNK, (p + 1) * PR * NCHUNK)
                tp_insts.append(nc.sync.dma_start_transpose(
                    out=xT[:, p * PR : (p + 1) * PR, :],
                    in_=xc_rows[hr, :],
                ))

            # Wo hi/lo splits: one big DMA each, after the x transposes.
            for wo_sb, wo_src, dep in (
                (wo_hi, wo_hi_d, tp_insts[-2]),
                (wo_lo, wo_lo_d, tp_insts[-1]),
            ):
                wi = nc.gpsimd.dma_start(
                    out=wo_sb,
                    in_=wo_src.rearrange("(h p) o -> p h o", p=128),
                )
                add_dep_helper(wi.ins, dep.ins, sync=True, reason="wo after x")

            with tc.tile_pool(name="lg_psum", bufs=1, space="PSUM") as lg_psum:
                # logits[n] = sum_k xT[:, n, k] . wtc   (wtc = (Wl@Wq)/16),
                # one accumulation group per transpose piece so PE starts as
                # soon as the first piece lands.
                lg = lg_psum.tile([1, N], F32)
                negrow = mrows.tile([1, N], F32)
                for p in range(NP):
                    for k in range(NCHUNK):
                        nc.tensor.matmul(
                            lg[:, p * PR : (p + 1) * PR],
                            wtc_sb,
                            xT[:, p * PR : (p + 1) * PR, k],
                            start=(k == 0),
                            stop=(k == NCHUNK - 1),
                        )
                    # negate per piece: runs on DVE while later pieces land
                    nc.vector.tensor_scalar_mul(
                        negrow[:, p * PR : (p + 1) * PR],
                        lg[:, p * PR : (p + 1) * PR], -1.0,
                    )

                # ---- mask: softmax over tokens, snap all but 25 smallest
                # to 1. DVE runs the serial top-k; Act computes the softmax
                # normalization in parallel.
                m8 = mrows.tile([1, 8], F32)
                for _ in range(3):
                    nc.vector.max(out=m8, in_=negrow)
                    nc.vector.match_replace(
                        out=negrow, in_to_replace=m8, in_values=negrow,
                        imm_value=NEG_BIG,
                    )
                nc.vector.max(out=m8, in_=negrow)  # 25th largest of -L
                thrneg = mrows.tile([1, 1], F32)
                nc.vector.tensor_scalar_mul(thrneg, m8[:, 0:1], -1.0)
                urow = mrows.tile([1, N], F32)
                ssum = mrows.tile([1, 1], F32)
                nc.scalar.activation(
                    out=urow, in_=lg, func=Exp, accum_out=ssum
                )
                srecip = mrows.tile([1, 1], F32)
                nc.vector.reciprocal(srecip, ssum)
                # normalize on DVE: on the (in-order) Act queue this would
                # block all phase-1 Q/K moves behind the srecip wait
                smrow = mrows.tile([1, N], F32)
                nc.vector.tensor_scalar_mul(smrow, urow, srecip)
                sel = mrows.tile([1, N], U32)
                nc.vector.tensor_scalar(
                    sel, lg, thrneg, None, op0=mybir.AluOpType.is_gt
                )
                onesrow = mrows.tile([1, N], F32)
                nc.vector.memset(onesrow, 1.0)
                nc.vector.copy_predicated(smrow, sel, onesrow)
                # k = sum(mask) - 1024 (~ -25 + tiny): V-bias fold + dn fixup
                msum = mrows.tile([1, 1], F32)
                mdummy = mrows.tile([1, N], F32)
                nc.vector.tensor_scalar(
                    mdummy, smrow, 1.0, 0.0, op0=mybir.AluOpType.mult,
                    op1=mybir.AluOpType.add, accum_out=msum,
                )
                ksc = mrows.tile([1, 1], F32)
                nc.vector.tensor_scalar_add(ksc, msum, -float(N))
                nc.vector.tensor_scalar_mul(ksc32, ksc, 1.0 / A_SCALE)

            # ---- Q/K/V projections, interleaved per head -----------------
            # Q/K produce transposed layouts [c', h, n]; V goes directly to
            # its PV-ready [token, channel] layout (x chunk as stationary).
            # V bias is handled via kbv + host bo fold; the V mask lives in
            # the exp bias (lnm_col), so V copies have no mask dependency.
            # Act paces the Q/K bias-moves; V copies go to DVE (idle once
            # the mask chain drains) except the last heads on Act.
            with tc.tile_pool(name="mc_psum", bufs=1, space="PSUM") as mc_psum:
                for h in range(HPC):
                    for jtg in range(2):
                        vp = v_psum.tile([128, 4, C], F32)
                        for dj in range(4):
                            jt = jtg * 4 + dj
                            nc.tensor.matmul(
                                vp[:, dj, :],
                                xT[:, jt * 128 : (jt + 1) * 128, h],
                                wv_sb,
                                start=True,
                                stop=True,
                            )
                        dst = vnat[:, h, jtg * 4 : (jtg + 1) * 4, :]
                        if h < 6:
                            nc.vector.tensor_copy(dst, vp)
                        else:
                            nc.scalar.activation(out=dst, in_=vp, func=Ident)
                    for w_sb, b_sb, dstT in ((wq_sb, bq_sb, qT), (wk_sb, bk_sb, kT)):
                        pp = mm_psum.tile([128, N], F32)
                        for half in range(2):
                            nc.tensor.matmul(
                                pp[:, half * 512 : (half + 1) * 512],
                                w_sb,
                                xT[:, half * 512 : (half + 1) * 512, h],
                                start=True,
                                stop=True,
                            )
                        nc.scalar.activation(
                            out=dstT[:, h, :], in_=pp, func=Ident, bias=b_sb
                        )
                    if h == 3:
                        # ---- mask row -> [128, 8] columns via tiny PE
                        # transposes (plus a [128,1] broadcast of k),
                        # replacing two DRAM bounces. Emitted mid-QKV: the
                        # PE is consumer-paced here (idle slots), smrow is
                        # ready by now, and scale_col/lnm_col stop gating
                        # phase 2's first exp.
                        mcol_ps = mc_psum.tile([128, 9], F32)
                        for t in range(8):
                            nc.tensor.transpose(
                                mcol_ps[:, t : t + 1],
                                smrow[:, t * 128 : (t + 1) * 128],
                                one_f32,
                            )
                        nc.tensor.matmul(
                            mcol_ps[:, 8:9], ones128_f32, ksc,
                            start=True, stop=True,
                        )
                        nc.vector.tensor_copy(mask_col, mcol_ps[:, 0:8])
                        nc.vector.tensor_scalar_mul(scale_col, mask_col, SCALE)
                        nc.vector.tensor_mul(kbv, mcol_ps[:, 8:9], bv_sb)
                    if h == 5:
                        # Ln deferred two heads: by now mask_col has landed,
                        # so this doesn't stall the Act queue mid-moves
                        nc.scalar.activation(
                            out=lnm_col, in_=mask_col, func=Ln
                        )

        # ================= phase 2: attention ==============================
        # Pool open order places dn/ot on the earliest-freed phase-1 banks.
        with (
            tc.tile_pool(name="pexp", bufs=2) as pexp_pool,
            tc.tile_pool(name="dvp", bufs=2) as dvp,
            tc.tile_pool(name="dn_psum", bufs=1, space="PSUM") as dn_psum,
            tc.tile_pool(name="ot_psum", bufs=1, space="PSUM") as ot_psum,
            tc.tile_pool(name="st_psum", bufs=2, space="PSUM") as st_psum,
        ):
            for h in range(HPC):
                ot = ot_psum.tile([128, N], F32, tag="ot")
                dn = dn_psum.tile([1, N], F32, tag="dn")
                pexp = pexp_pool.tile([128, 8, N], BF16, tag="pexp")

                def emit_pvdn(jt, ot=ot, dn=dn, pexp=pexp, h=h):
                    for half in range(2):
                        sl = slice(half * 512, (half + 1) * 512)
                        nc.tensor.matmul(
                            ot[:, sl],
                            vnat[:, h, jt, :],
                            pexp[:, jt, sl],
                            start=(jt == 0),
                            stop=(jt == 7),
                        )
                        nc.tensor.matmul(
                            dn[:, sl],
                            ones32,
                            pexp[:, jt, sl],
                            start=(jt == 0),
                            stop=(jt == 7),
                        )

                pending = None
                for jt in range(8):
                    st = st_psum.tile([128, N], F32, tag="st")
                    for half in range(2):
                        nc.tensor.matmul(
                            st[:, half * 512 : (half + 1) * 512],
                            kT[:, h, jt * 128 : (jt + 1) * 128],
                            qT[:, h, half * 512 : (half + 1) * 512],
                            start=True,
                            stop=True,
                        )
                    nc.scalar.activation(
                        out=pexp[:, jt, :], in_=st, func=Exp,
                        scale=scale_col[:, jt : jt + 1],
                        bias=lnm_col[:, jt : jt + 1],
                    )
                    if pending is not None:
                        emit_pvdn(pending)
                    pending = jt
                emit_pvdn(pending)

                # drain: otsb = u + k*bv (DVE per-partition add, frees ot);
                # dn' holds sum_j m_j p_j / 32, true dn/32 = dn' - k/32 (the
                # 25 masked tokens have p ~= 1); tmp = otsb * (32/dn);
                # hi/lo fp8 split of tmp.
                otsb = dvp.tile([128, N], BF16, tag="otsb")
                if h < HPC - 1:
                    nc.vector.tensor_scalar(
                        otsb, ot, kbv, None, op0=mybir.AluOpType.add
                    )
                else:
                    # last head: Act is idle here; keep DVE free for the
                    # to_out-gating dnadj/recip/tmp/hi/lo chain
                    nc.scalar.activation(
                        out=otsb, in_=ot, func=Ident, bias=kbv
                    )
                dnadj = dvp.tile([1, N], F32, tag="dnadj")
                nc.vector.tensor_scalar(
                    dnadj, dn, ksc32, None, op0=mybir.AluOpType.subtract
                )
                if h < HPC - 1:
                    rrow = dvp.tile([1, N], F32, tag="rrow")
                    nc.vector.reciprocal(rrow, dnadj)
                    w_i = nc.sync.dma_start(out=dscr[h, :], in_=rrow)
                    rb = dvp.tile([128, N], F32, tag="rb")
                    r_i = nc.sync.dma_start(
                        out=rb, in_=dscr[h, :].partition_broadcast(128)
                    )
                    add_dep_helper(r_i.ins, w_i.ins, sync=True,
                                   reason="recip RAW")
                else:
                    # last head gates to_out: broadcast 1/dn across partitions
                    # with a K=1 matmul into the freed ot slot instead of the
                    # (slower) DRAM round-trip.
                    rrow_bf = dvp.tile([1, N], BF16, tag="rrowbf")
                    with nc.allow_low_precision(
                        reason="1/dn broadcast row; 0.4% relative is fine"
                    ):
                        nc.vector.reciprocal(rrow_bf, dnadj)
                    rb = ot_psum.tile([128, N], F32, tag="ot")
                    for half in range(2):
                        sl = slice(half * 512, (half + 1) * 512)
                        nc.tensor.matmul(
                            rb[:, sl], ones128_bf, rrow_bf[:, sl],
                            start=True, stop=True,
                        )
                tmp = dvp.tile([128, N], BF16, tag="tmp")
                nc.vector.tensor_mul(tmp, otsb, rb)
                nc.vector.tensor_copy(outT_hi[:, h, :], tmp)
                nc.vector.tensor_sub(
                    outT_lo[:, h, :], tmp, outT_hi[:, h, :]
                )

            # ============= phase 4: to_out partial (fp8 DoubleRow) =========
            # fo shares the st_psum slots (same shape) so Wo accumulation can
            # begin as soon as the last exp frees an ST slot.
            with tc.tile_pool(name="fout", bufs=3) as fout_pool:
                PRODUCTS = ((wo_hi, outT_hi), (wo_hi, outT_lo), (wo_lo, outT_hi))

                def finish_oc(oc, fo):
                    fout = fout_pool.tile([128, N], BF16)
                    eng = nc.sync if oc % 2 == 0 else nc.gpsimd
                    if oc < 15:
                        nc.vector.tensor_scalar_mul(
                            fout, fo, 1.0 / (WO_SCALE * A_SCALE)
                        )
                        eng.dma_start(
                            out=outT_d[oc * 128 : (oc + 1) * 128, :], in_=fout
                        )
                    else:
                        # final oc: drain per half so the tail DMA covers
                        # only 512 columns
                        for sh in range(2):
                            ssl = slice(sh * 512, (sh + 1) * 512)
                            nc.vector.tensor_scalar_mul(
                                fout[:, ssl], fo[:, ssl],
                                1.0 / (WO_SCALE * A_SCALE),
                            )
                            eng = nc.sync if sh == 0 else nc.gpsimd
                            eng.dma_start(
                                out=outT_d[oc * 128 : (oc + 1) * 128, ssl],
                                in_=fout[:, ssl],
                            )

                pending_oc = None
                for oc in range(16):
                    fo = st_psum.tile([128, N], F32, tag="st")
                    for half in range(2):
                        sl = slice(half * 512, (half + 1) * 512)
                        # t=3 (heads 6,7) last in every product so the first
                        # 9 instructions don't wait on head 7's drain chain
                        steps = [(w, a, t) for t in (0, 1, 2)
                                 for (w, a) in PRODUCTS]
                        steps += [(w, a, 3) for (w, a) in PRODUCTS]
                        for i, (wsp, asp, t) in enumerate(steps):
                            nc.tensor.matmul(
                                fo[:, sl],
                                wsp[:, 2 * t : 2 * t + 2,
                                    oc * 128 : (oc + 1) * 128],
                                asp[:, 2 * t : 2 * t + 2, sl],
                                start=(i == 0),
                                stop=(i == len(steps) - 1),
                                perf_mode=DR,
                            )
                    if pending_oc is not None:
                        finish_oc(*pending_oc)
                    pending_oc = (oc, fo)
                finish_oc(*pending_oc)


_CACHE = {}


def _get_module():
    if "nc" in _CACHE:
        return _CACHE["nc"]
    nc = bacc.Bacc("TRN2", target_bir_lowering=False, debug=False, num_devices=8)
    xc = nc.dram_tensor("xc", (N, D), BF16, kind="ExternalInput").ap()
    wq_d = nc.dram_tensor("wqkvT", (C, 3 * C), BF16, kind="ExternalInput").ap()
    bq_d = nc.dram_tensor("bqkv", (C, 3), F32, kind="ExternalInput").ap()
    wtc_d = nc.dram_tensor("wtc", (C, 1), BF16, kind="ExternalInput").ap()
    wo_hi_d = nc.dram_tensor("woHi", (HPC * C, D), FP8, kind="ExternalInput").ap()
    wo_lo_d = nc.dram_tensor("woLo", (HPC * C, D), FP8, kind="ExternalInput").ap()
    outT_d = nc.dram_tensor("outT", (D, N), BF16, kind="ExternalOutput").ap()

    with tile.TileContext(nc) as tc:
        _body(tc, xc, wq_d, bq_d, wtc_d, wo_hi_d, wo_lo_d, outT_d)
    nc.compile()
    _CACHE["nc"] = nc
    return nc


def make_in_maps(x, Wq, bq, Wk, bk, Wv, bv, Wl, bl, Wo, bo):
    bf16 = ml_dtypes.bfloat16
    fp8 = ml_dtypes.float8_e4m3
    x = np.ascontiguousarray(np.asarray(x, np.float32))
    Wq = np.asarray(Wq, np.float32)
    Wk = np.asarray(Wk, np.float32)
    Wv = np.asarray(Wv, np.float32)
    Wl = np.asarray(Wl, np.float32)
    Wo = np.asarray(Wo, np.float32)
    we = (Wl[0] @ Wq) / float(NCHUNK)  # (128,) logits weight per chunk
    common = {
        "wqkvT": np.ascontiguousarray(
            np.concatenate([Wq.T, Wk.T, Wv.T], axis=1)
        ).astype(bf16),
        "bqkv": np.ascontiguousarray(np.stack(
            [np.asarray(bq, np.float32), np.asarray(bk, np.float32),
             np.asarray(bv, np.float32)], axis=1
        )),
        "wtc": we.astype(bf16).reshape(C, 1),
    }
    woT = np.ascontiguousarray(Wo.T) * WO_SCALE  # (d, o), prescaled
    in_maps = []
    halves = {}
    for g in range(2):
        wh = woT[g * 1024 : (g + 1) * 1024, :]
        hi = wh.astype(fp8)
        lo = (wh - hi.astype(np.float32)).astype(fp8)
        halves[g] = (np.ascontiguousarray(hi), np.ascontiguousarray(lo))
    for core in range(8):
        b, g = divmod(core, 2)
        xb = x[b]
        xcore = xb if g == 0 else np.concatenate(
            [xb[:, 1024:], xb[:, :1024]], axis=1
        )
        in_maps.append({
            "xc": np.ascontiguousarray(xcore.astype(bf16)),
            "woHi": halves[g][0],
            "woLo": halves[g][1],
            **common,
        })
    return in_maps


def run_spmd(in_maps, trace=False, **kw):
    nc = _get_module()
    return bass_utils.run_bass_kernel_spmd(
        nc, in_maps, core_ids=list(range(8)), trace=trace, **kw
    )


def gather(results, Wo, bv, bo):
    Wo = np.asarray(Wo, np.float32)
    bv = np.asarray(bv, np.float32)
    bo = np.asarray(bo, np.float32)
    # a = a_tilde + bv per head-channel: fold bv through Wo into the bias.
    bo_eff = bo + np.tile(bv, NCHUNK) @ Wo.T
    out = np.empty((B, N, D), np.float32)
    for b in range(B):
        p0 = results[2 * b]["outT"].astype(np.float32).T
        p1 = results[2 * b + 1]["outT"].astype(np.float32).T
        out[b] = p0 + p1 + bo_eff
    return out


def kernel(x, Wq, bq, Wk, bk, Wv, bv, Wl, bl, Wo, bo, stage=None, **_unused):
    in_maps = make_in_maps(x, Wq, bq, Wk, bk, Wv, bv, Wl, bl, Wo, bo)
    try:
        res = run_spmd(in_maps)
    except Exception:
        # transient device/runtime hiccup: retry once after a short pause
        import time as _time

        _time.sleep(2.0)
        res = run_spmd(in_maps)
    return gather(res.results, Wo, bv, bo)


# revision 51
# speedup vs baseline: 1.0646x; 1.0333x over previous
"""Trainium2 Bass kernel for nn_Attention_54614804136573 (topk_masking).

Sharding: 8 cores = 4 batches x 2 head-groups (8 heads each). Each core gets
its batch's full x (columns rotated so its own 8 head-chunks come first),
computes the token-importance mask redundantly, runs its 8 heads of attention,
and produces a partial to_out product over its 1024-wide d-slice for all 2048
output channels. The host sums the two partials per batch and adds bo'
(bo with the V-bias term folded in).

Key structure vs the previous version:
- x arrives as bf16 and is transposed into SBUF chunk-wise by the DMA XBAR
  (dma_start_transpose), removing all PE transposes and PSUM copies for x.
- V is projected directly into its PV-ready [token, channel] layout by using
  the x chunk as the stationary operand (out = xT_chunk.T @ WvT), removing
  the separate V transpose pass. The V bias is exactly handled outside the
  kernel: a = u/dn + bv*(S_pm/dn) with S_pm ~= dn + (sum(mask)-1024), so the
  per-head bias folds into an Act bias (k*bv) plus a host-side bo term.
- The softmax denominator is an M=1 ones-matmul accumulated over all 8
  j-tiles into a [1, N] PSUM row (ones = 1/32 so the normalized output is
  pre-scaled by 32 for fp8).
- to_out runs in fp8 DoubleRow (K=256/instr at 0.5 cycles/row) with an
  error-compensated hi+lo split of both Wo (host side, x64) and the
  attention output (device side, x32): w*a ~= w_hi*a_hi + w_hi*a_lo +
  w_lo*a_hi, 12 DR matmuls per (oc, half) instead of 16 bf16 matmuls.
"""

import sys

sys.path.insert(0, "/opt/trn_rl_repo")

import numpy as np
import ml_dtypes

import concourse.mybir as mybir
import concourse.tile as tile
from concourse import bacc, bass_utils
from concourse.tile import add_dep_helper

B = 4
N = 1024
C = 128
D = 2048
NCHUNK = 16  # d-chunks of 128 (= patch positions = heads)
HPC = 8  # heads per core
MASK_NUM = 25
SCALE = 64.0 ** -0.5  # 0.125

F32 = mybir.dt.float32
BF16 = mybir.dt.bfloat16
FP8 = mybir.dt.float8e4
U32 = mybir.dt.uint32
Exp = mybir.ActivationFunctionType.Exp
Ident = mybir.ActivationFunctionType.Identity
Ln = mybir.ActivationFunctionType.Ln
DR = mybir.MatmulPerfMode.DoubleRow
NEG_BIG = -1e30

WO_SCALE = 64.0  # host-side Wo prescale before fp8 split
A_SCALE = 32.0   # device-side attention-out prescale (via ones = 1/32)


def _body(tc, xc, wq_d, bq_d, wtc_d, wo_hi_d, wo_lo_d, outT_d):
    nc = tc.nc
    dscr = nc.dram_tensor("dscr", (HPC, N), F32, kind="Internal").ap()

    with (
        tc.tile_pool(name="consts", bufs=1) as consts,
        tc.tile_pool(name="persist", bufs=1) as persist,
    ):
        # ---- constants ----
        ones32 = consts.tile([128, 1], BF16)
        nc.vector.memset(ones32, 1.0 / A_SCALE)
        one_f32 = consts.tile([1, 1], F32)
        nc.vector.memset(one_f32, 1.0)
        ones128_f32 = consts.tile([1, 128], F32)
        nc.vector.memset(ones128_f32, 1.0)
        ones128_bf = consts.tile([1, 128], BF16)
        nc.vector.memset(ones128_bf, 1.0)

        # ---- persistent activations ----
        qT = persist.tile([128, HPC, N], BF16)      # [c', h, n] 2 MB
        kT = persist.tile([128, HPC, N], BF16)      # 2 MB
        vnat = persist.tile([128, HPC, 8, C], BF16)  # [j, h, jt, c] 2 MB
        outT_hi = persist.tile([128, HPC, N], FP8)  # 32*(a - bv) hi split
        outT_lo = persist.tile([128, HPC, N], FP8)
        wo_hi = persist.tile([128, HPC, D], FP8)    # [d, h-chunk, o] 2 MB
        wo_lo = persist.tile([128, HPC, D], FP8)
        mask_col = persist.tile([128, 8], F32)
        scale_col = persist.tile([128, 8], F32)
        lnm_col = persist.tile([128, 8], F32)       # ln(mask) exp bias
        kbv = persist.tile([128, 1], F32)           # (sum(m)-1024) * bv
        ksc32 = persist.tile([1, 1], F32)           # (sum(m)-1024)/32

        # ============ phase 1: x transpose-in, logits, mask, QKV ===========
        with (
            tc.tile_pool(name="ph1big", bufs=1) as ph1big,
            tc.tile_pool(name="mrows", bufs=1) as mrows,
            tc.tile_pool(name="mm_psum", bufs=2, space="PSUM") as mm_psum,
            tc.tile_pool(name="v_psum", bufs=2, space="PSUM") as v_psum,
        ):
            # packed weight loads first (tiny; scalar queue)
            wqkv_sb = consts.tile([C, 3 * C], BF16)
            nc.scalar.dma_start(out=wqkv_sb, in_=wq_d)
            wq_sb = wqkv_sb[:, 0:C]
            wk_sb = wqkv_sb[:, C : 2 * C]
            wv_sb = wqkv_sb[:, 2 * C : 3 * C]
            bqkv_sb = consts.tile([C, 3], F32)
            nc.scalar.dma_start(out=bqkv_sb, in_=bq_d)
            bq_sb = bqkv_sb[:, 0:1]
            bk_sb = bqkv_sb[:, 1:2]
            bv_sb = bqkv_sb[:, 2:3]
            wtc_sb = consts.tile([C, 1], BF16)
            nc.scalar.dma_start(out=wtc_sb, in_=wtc_d)

            # x transposed in by the DMA XBAR in 8 strided sweeps (pipelined
            # so logits can chase them). xc rows viewed as [(n k), c] with
            # row-stride 256B give layout xT[c, n, k] (k fastest on free).
            NP = 8
            PR = N // NP  # 128 tokens per piece
            xT = ph1big.tile([128, N, NCHUNK], BF16)  # [c, n, k] 4 MB
            xc_rows = xc.rearrange("n (k c) -> (n k) c", c=128)
            tp_insts = []
            for p in range(NP):
                hr = slice(p * PR * NCHUNK, (p + 1) * PR * NCHUNK)
                tp_insts.append(nc.sync.dma_start_transpose(
                    out=xT[:, p * PR : (p + 1) * PR, :],
                    in_=xc_rows[hr, :],
                ))

            # Wo hi/lo splits: one big DMA each, after the x transposes.
            for wo_sb, wo_src, dep in (
                (wo_hi, wo_hi_d, tp_insts[-2]),
                (wo_lo, wo_lo_d, tp_insts[-1]),
            ):
                wi = nc.gpsimd.dma_start(
                    out=wo_sb,
                    in_=wo_src.rearrange("(h p) o -> p h o", p=128),
                )
                add_dep_helper(wi.ins, dep.ins, sync=True, reason="wo after x")

            with tc.tile_pool(name="lg_psum", bufs=1, space="PSUM") as lg_psum:
                # logits[n] = sum_k xT[:, n, k] . wtc   (wtc = (Wl@Wq)/16),
                # one accumulation group per transpose piece so PE starts as
                # soon as the first piece lands.
                lg = lg_psum.tile([1, N], F32)
                negrow = mrows.tile([1, N], F32)
                for p in range(NP):
                    for k in range(NCHUNK):
                        nc.tensor.matmul(
                            lg[:, p * PR : (p + 1) * PR],
                            wtc_sb,
                            xT[:, p * PR : (p + 1) * PR, k],
                            start=(k == 0),
                            stop=(k == NCHUNK - 1),
                        )
                    # negate per piece: runs on DVE while later pieces land
                    nc.vector.tensor_scalar_mul(
                        negrow[:, p * PR : (p + 1) * PR],
                        lg[:, p * PR : (p + 1) * PR], -1.0,
                    )

                # ---- mask: softmax over tokens, snap all but 25 smallest
                # to 1. DVE runs the serial top-k; Act computes the softmax
                # normalization in parallel.
                m8 = mrows.tile([1, 8], F32)
                for _ in range(3):
                    nc.vector.max(out=m8, in_=negrow)
                    nc.vector.match_replace(
                        out=negrow, in_to_replace=m8, in_values=negrow,
                        imm_value=NEG_BIG,
                    )
                nc.vector.max(out=m8, in_=negrow)  # 25th largest of -L
                thrneg = mrows.tile([1, 1], F32)
                nc.vector.tensor_scalar_mul(thrneg, m8[:, 0:1], -1.0)
                urow = mrows.tile([1, N], F32)
                ssum = mrows.tile([1, 1], F32)
                nc.scalar.activation(
                    out=urow, in_=lg, func=Exp, accum_out=ssum
                )
                srecip = mrows.tile([1, 1], F32)
                nc.vector.reciprocal(srecip, ssum)
                # normalize on DVE: on the (in-order) Act queue this would
                # block all phase-1 Q/K moves behind the srecip wait
                smrow = mrows.tile([1, N], F32)
                nc.vector.tensor_scalar_mul(smrow, urow, srecip)
                sel = mrows.tile([1, N], U32)
                nc.vector.tensor_scalar(
                    sel, lg, thrneg, None, op0=mybir.AluOpType.is_gt
                )
                onesrow = mrows.tile([1, N], F32)
                nc.vector.memset(onesrow, 1.0)
                nc.vector.copy_predicated(smrow, sel, onesrow)
                # k = sum(mask) - 1024 (~ -25 + tiny): V-bias fold + dn fixup
                msum = mrows.tile([1, 1], F32)
                mdummy = mrows.tile([1, N], F32)
                nc.vector.tensor_scalar(
                    mdummy, smrow, 1.0, 0.0, op0=mybir.AluOpType.mult,
                    op1=mybir.AluOpType.add, accum_out=msum,
                )
                ksc = mrows.tile([1, 1], F32)
                nc.vector.tensor_scalar_add(ksc, msum, -float(N))
                nc.vector.tensor_scalar_mul(ksc32, ksc, 1.0 / A_SCALE)

            # ---- Q/K/V projections, interleaved per head -----------------
            # Q/K produce transposed layouts [c', h, n]; V goes directly to
            # its PV-ready [token, channel] layout (x chunk as stationary).
            # V bias is handled via kbv + host bo fold; the V mask lives in
            # the exp bias (lnm_col), so V copies have no mask dependency.
            # Act paces the Q/K bias-moves; V copies go to DVE (idle once
            # the mask chain drains) except the last heads on Act.
            with tc.tile_pool(name="mc_psum", bufs=1, space="PSUM") as mc_psum:
                for h in range(HPC):
                    for jtg in range(2):
                        vp = v_psum.tile([128, 4, C], F32)
                        for dj in range(4):
                            jt = jtg * 4 + dj
                            nc.tensor.matmul(
                                vp[:, dj, :],
                                xT[:, jt * 128 : (jt + 1) * 128, h],
                                wv_sb,
                                start=True,
                                stop=True,
                            )
                        dst = vnat[:, h, jtg * 4 : (jtg + 1) * 4, :]
                        # all V copies on DVE: the Act queue tail (urow +
                        # 16 Q/K moves) is the phase-2 gate, DVE has slack
                        nc.vector.tensor_copy(dst, vp)
                    for w_sb, b_sb, dstT in ((wq_sb, bq_sb, qT), (wk_sb, bk_sb, kT)):
                        pp = mm_psum.tile([128, N], F32)
                        for half in range(2):
                            nc.tensor.matmul(
                                pp[:, half * 512 : (half + 1) * 512],
                                w_sb,
                                xT[:, half * 512 : (half + 1) * 512, h],
                                start=True,
                                stop=True,
                            )
                        nc.scalar.activation(
                            out=dstT[:, h, :], in_=pp, func=Ident, bias=b_sb
                        )
                    if h == 3:
                        # ---- mask row -> [128, 8] columns via tiny PE
                        # transposes (plus a [128,1] broadcast of k),
                        # replacing two DRAM bounces. Emitted mid-QKV: the
                        # PE is consumer-paced here (idle slots), smrow is
                        # ready by now, and scale_col/lnm_col stop gating
                        # phase 2's first exp.
                        mcol_ps = mc_psum.tile([128, 9], F32)
                        for t in range(8):
                            nc.tensor.transpose(
                                mcol_ps[:, t : t + 1],
                                smrow[:, t * 128 : (t + 1) * 128],
                                one_f32,
                            )
                        nc.tensor.matmul(
                            mcol_ps[:, 8:9], ones128_f32, ksc,
                            start=True, stop=True,
                        )
                        nc.vector.tensor_copy(mask_col, mcol_ps[:, 0:8])
                        nc.vector.tensor_scalar_mul(scale_col, mask_col, SCALE)
                        nc.vector.tensor_mul(kbv, mcol_ps[:, 8:9], bv_sb)
                    if h == 7:
                        # Ln deferred past all Q/K moves: it waits on
                        # mask_col, and anywhere earlier it stalls the
                        # in-order Act queue ahead of the remaining moves
                        nc.scalar.activation(
                            out=lnm_col, in_=mask_col, func=Ln
                        )

        # ================= phase 2: attention ==============================
        # Pool open order places dn/ot on the earliest-freed phase-1 banks.
        with (
            tc.tile_pool(name="pexp", bufs=2) as pexp_pool,
            tc.tile_pool(name="dvp", bufs=2) as dvp,
            tc.tile_pool(name="dn_psum", bufs=1, space="PSUM") as dn_psum,
            tc.tile_pool(name="ot_psum", bufs=1, space="PSUM") as ot_psum,
            tc.tile_pool(name="st_psum", bufs=2, space="PSUM") as st_psum,
        ):
            for h in range(HPC):
                ot = ot_psum.tile([128, N], F32, tag="ot")
                dn = dn_psum.tile([1, N], F32, tag="dn")
                pexp = pexp_pool.tile([128, 8, N], BF16, tag="pexp")

                def emit_pvdn(jt, ot=ot, dn=dn, pexp=pexp, h=h):
                    for half in range(2):
                        sl = slice(half * 512, (half + 1) * 512)
                        nc.tensor.matmul(
                            ot[:, sl],
                            vnat[:, h, jt, :],
                            pexp[:, jt, sl],
                            start=(jt == 0),
                            stop=(jt == 7),
                        )
                        nc.tensor.matmul(
                            dn[:, sl],
                            ones32,
                            pexp[:, jt, sl],
                            start=(jt == 0),
                            stop=(jt == 7),
                        )

                pending = None
                for jt in range(8):
                    st = st_psum.tile([128, N], F32, tag="st")
                    for half in range(2):
                        nc.tensor.matmul(
                            st[:, half * 512 : (half + 1) * 512],
                            kT[:, h, jt * 128 : (jt + 1) * 128],
                            qT[:, h, half * 512 : (half + 1) * 512],
                            start=True,
                            stop=True,
                        )
                    nc.scalar.activation(
                        out=pexp[:, jt, :], in_=st, func=Exp,
                        scale=scale_col[:, jt : jt + 1],
                        bias=lnm_col[:, jt : jt + 1],
                    )
                    if pending is not None:
                        emit_pvdn(pending)
                    pending = jt
                emit_pvdn(pending)

                # drain: otsb = u + k*bv (DVE per-partition add, frees ot);
                # dn' holds sum_j m_j p_j / 32, true dn/32 = dn' - k/32 (the
                # 25 masked tokens have p ~= 1); tmp = otsb * (32/dn);
                # hi/lo fp8 split of tmp.
                otsb = dvp.tile([128, N], BF16, tag="otsb")
                if h < HPC - 1:
                    nc.vector.tensor_scalar(
                        otsb, ot, kbv, None, op0=mybir.AluOpType.add
                    )
                else:
                    # last head: Act is idle here; keep DVE free for the
                    # to_out-gating dnadj/recip/tmp/hi/lo chain
                    nc.scalar.activation(
                        out=otsb, in_=ot, func=Ident, bias=kbv
                    )
                dnadj = dvp.tile([1, N], F32, tag="dnadj")
                nc.vector.tensor_scalar(
                    dnadj, dn, ksc32, None, op0=mybir.AluOpType.subtract
                )
                if h < HPC - 1:
                    rrow = dvp.tile([1, N], F32, tag="rrow")
                    nc.vector.reciprocal(rrow, dnadj)
                    w_i = nc.sync.dma_start(out=dscr[h, :], in_=rrow)
                    rb = dvp.tile([128, N], F32, tag="rb")
                    r_i = nc.sync.dma_start(
                        out=rb, in_=dscr[h, :].partition_broadcast(128)
                    )
                    add_dep_helper(r_i.ins, w_i.ins, sync=True,
                                   reason="recip RAW")
                else:
                    # last head gates to_out: broadcast 1/dn across partitions
                    # with a K=1 matmul into the freed ot slot instead of the
                    # (slower) DRAM round-trip.
                    rrow_bf = dvp.tile([1, N], BF16, tag="rrowbf")
                    with nc.allow_low_precision(
                        reason="1/dn broadcast row; 0.4% relative is fine"
                    ):
                        nc.vector.reciprocal(rrow_bf, dnadj)
                    rb = ot_psum.tile([128, N], F32, tag="ot")
                    for half in range(2):
                        sl = slice(half * 512, (half + 1) * 512)
                        nc.tensor.matmul(
                            rb[:, sl], ones128_bf, rrow_bf[:, sl],
                            start=True, stop=True,
                        )
                tmp = dvp.tile([128, N], BF16, tag="tmp")
                nc.vector.tensor_mul(tmp, otsb, rb)
                nc.vector.tensor_copy(outT_hi[:, h, :], tmp)
                nc.vector.tensor_sub(
                    outT_lo[:, h, :], tmp, outT_hi[:, h, :]
                )

            # ============= phase 4: to_out partial (fp8 DoubleRow) =========
            # fo shares the st_psum slots (same shape) so Wo accumulation can
            # begin as soon as the last exp frees an ST slot.
            with tc.tile_pool(name="fout", bufs=3) as fout_pool:
                PRODUCTS = ((wo_hi, outT_hi), (wo_hi, outT_lo), (wo_lo, outT_hi))

                def finish_oc(oc, fo):
                    fout = fout_pool.tile([128, N], BF16)
                    eng = nc.sync if oc % 2 == 0 else nc.gpsimd
                    if oc < 15:
                        nc.vector.tensor_scalar_mul(
                            fout, fo, 1.0 / (WO_SCALE * A_SCALE)
                        )
                        eng.dma_start(
                            out=outT_d[oc * 128 : (oc + 1) * 128, :], in_=fout
                        )
                    else:
                        # final oc: drain per half so the tail DMA covers
                        # only 512 columns
                        for sh in range(2):
                            ssl = slice(sh * 512, (sh + 1) * 512)
                            nc.vector.tensor_scalar_mul(
                                fout[:, ssl], fo[:, ssl],
                                1.0 / (WO_SCALE * A_SCALE),
                            )
                            eng = nc.sync if sh == 0 else nc.gpsimd
                            eng.dma_start(
                                out=outT_d[oc * 128 : (oc + 1) * 128, ssl],
                                in_=fout[:, ssl],
                            )

                pending_oc = None
                for oc in range(16):
                    fo = st_psum.tile([128, N], F32, tag="st")
                    for half in range(2):
                        sl = slice(half * 512, (half + 1) * 512)
                        # t=3 (heads 6,7) last in every product so the first
                        # 9 instructions don't wait on head 7's drain chain
                        steps = [(w, a, t) for t in (0, 1, 2)
                                 for (w, a) in PRODUCTS]
                        steps += [(w, a, 3) for (w, a) in PRODUCTS]
                        for i, (wsp, asp, t) in enumerate(steps):
                            nc.tensor.matmul(
                                fo[:, sl],
                                wsp[:, 2 * t : 2 * t + 2,
                                    oc * 128 : (oc + 1) * 128],
                                asp[:, 2 * t : 2 * t + 2, sl],
                                start=(i == 0),
                                stop=(i == len(steps) - 1),
                                perf_mode=DR,
                            )
                    if pending_oc is not None:
                        finish_oc(*pending_oc)
                    pending_oc = (oc, fo)
                finish_oc(*pending_oc)


_CACHE = {}


def _get_module():
    if "nc" in _CACHE:
        return _CACHE["nc"]
    nc = bacc.Bacc("TRN2", target_bir_lowering=False, debug=False, num_devices=8)
    xc = nc.dram_tensor("xc", (N, D), BF16, kind="ExternalInput").ap()
    wq_d = nc.dram_tensor("wqkvT", (C, 3 * C), BF16, kind="ExternalInput").ap()
    bq_d = nc.dram_tensor("bqkv", (C, 3), F32, kind="ExternalInput").ap()
    wtc_d = nc.dram_tensor("wtc", (C, 1), BF16, kind="ExternalInput").ap()
    wo_hi_d = nc.dram_tensor("woHi", (HPC * C, D), FP8, kind="ExternalInput").ap()
    wo_lo_d = nc.dram_tensor("woLo", (HPC * C, D), FP8, kind="ExternalInput").ap()
    outT_d = nc.dram_tensor("outT", (D, N), BF16, kind="ExternalOutput").ap()

    with tile.TileContext(nc) as tc:
        _body(tc, xc, wq_d, bq_d, wtc_d, wo_hi_d, wo_lo_d, outT_d)
    nc.compile()
    _CACHE["nc"] = nc
    return nc


def make_in_maps(x, Wq, bq, Wk, bk, Wv, bv, Wl, bl, Wo, bo):
    bf16 = ml_dtypes.bfloat16
    fp8 = ml_dtypes.float8_e4m3
    x = np.ascontiguousarray(np.asarray(x, np.float32))
    Wq = np.asarray(Wq, np.float32)
    Wk = np.asarray(Wk, np.float32)
    Wv = np.asarray(Wv, np.float32)
    Wl = np.asarray(Wl, np.float32)
    Wo = np.asarray(Wo, np.float32)
    we = (Wl[0] @ Wq) / float(NCHUNK)  # (128,) logits weight per chunk
    common = {
        "wqkvT": np.ascontiguousarray(
            np.concatenate([Wq.T, Wk.T, Wv.T], axis=1)
        ).astype(bf16),
        "bqkv": np.ascontiguousarray(np.stack(
            [np.asarray(bq, np.float32), np.asarray(bk, np.float32),
             np.asarray(bv, np.float32)], axis=1
        )),
        "wtc": we.astype(bf16).reshape(C, 1),
    }
    woT = np.ascontiguousarray(Wo.T) * WO_SCALE  # (d, o), prescaled
    in_maps = []
    halves = {}
    for g in range(2):
        wh = woT[g * 1024 : (g + 1) * 1024, :]
        hi = wh.astype(fp8)
        lo = (wh - hi.astype(np.float32)).astype(fp8)
        halves[g] = (np.ascontiguousarray(hi), np.ascontiguousarray(lo))
    for core in range(8):
        b, g = divmod(core, 2)
        xb = x[b]
        xcore = xb if g == 0 else np.concatenate(
            [xb[:, 1024:], xb[:, :1024]], axis=1
        )
        in_maps.append({
            "xc": np.ascontiguousarray(xcore.astype(bf16)),
            "woHi": halves[g][0],
            "woLo": halves[g][1],
            **common,
        })
    return in_maps


def run_spmd(in_maps, trace=False, **kw):
    nc = _get_module()
    return bass_utils.run_bass_kernel_spmd(
        nc, in_maps, core_ids=list(range(8)), trace=trace, **kw
    )


def gather(results, Wo, bv, bo):
    Wo = np.asarray(Wo, np.float32)
    bv = np.asarray(bv, np.float32)
    bo = np.asarray(bo, np.float32)
    # a = a_tilde + bv per head-channel: fold bv through Wo into the bias.
    bo_eff = bo + np.tile(bv, NCHUNK) @ Wo.T
    out = np.empty((B, N, D), np.float32)
    for b in range(B):
        p0 = results[2 * b]["outT"].astype(np.float32).T
        p1 = results[2 * b + 1]["outT"].astype(np.float32).T
        out[b] = p0 + p1 + bo_eff
    return out


def kernel(x, Wq, bq, Wk, bk, Wv, bv, Wl, bl, Wo, bo, stage=None, **_unused):
    in_maps = make_in_maps(x, Wq, bq, Wk, bk, Wv, bv, Wl, bl, Wo, bo)
    try:
        res = run_spmd(in_maps)
    except Exception:
        # transient device/runtime hiccup: retry once after a short pause
        import time as _time

        _time.sleep(2.0)
        res = run_spmd(in_maps)
    return gather(res.results, Wo, bv, bo)


# revision 53
# speedup vs baseline: 1.0709x; 1.0059x over previous
"""Trainium2 Bass kernel for nn_Attention_54614804136573 (topk_masking).

Sharding: 8 cores = 4 batches x 2 head-groups (8 heads each). Each core gets
its batch's full x (columns rotated so its own 8 head-chunks come first),
computes the token-importance mask redundantly, runs its 8 heads of attention,
and produces a partial to_out product over its 1024-wide d-slice for all 2048
output channels. The host sums the two partials per batch and adds bo'
(bo with the V-bias term folded in).

Key structure vs the previous version:
- x arrives as bf16 and is transposed into SBUF chunk-wise by the DMA XBAR
  (dma_start_transpose), removing all PE transposes and PSUM copies for x.
- V is projected directly into its PV-ready [token, channel] layout by using
  the x chunk as the stationary operand (out = xT_chunk.T @ WvT), removing
  the separate V transpose pass. The V bias is exactly handled outside the
  kernel: a = u/dn + bv*(S_pm/dn) with S_pm ~= dn + (sum(mask)-1024), so the
  per-head bias folds into an Act bias (k*bv) plus a host-side bo term.
- The softmax denominator is an M=1 ones-matmul accumulated over all 8
  j-tiles into a [1, N] PSUM row (ones = 1/32 so the normalized output is
  pre-scaled by 32 for fp8).
- to_out runs in fp8 DoubleRow (K=256/instr at 0.5 cycles/row) with an
  error-compensated hi+lo split of both Wo (host side, x64) and the
  attention output (device side, x32): w*a ~= w_hi*a_hi + w_hi*a_lo +
  w_lo*a_hi, 12 DR matmuls per (oc, half) instead of 16 bf16 matmuls.
"""

import sys

sys.path.insert(0, "/opt/trn_rl_repo")

import numpy as np
import ml_dtypes

import concourse.mybir as mybir
import concourse.tile as tile
from concourse import bacc, bass_utils
from concourse.tile import add_dep_helper

B = 4
N = 1024
C = 128
D = 2048
NCHUNK = 16  # d-chunks of 128 (= patch positions = heads)
HPC = 8  # heads per core
MASK_NUM = 25
SCALE = 64.0 ** -0.5  # 0.125

F32 = mybir.dt.float32
BF16 = mybir.dt.bfloat16
FP8 = mybir.dt.float8e4
U32 = mybir.dt.uint32
Exp = mybir.ActivationFunctionType.Exp
Ident = mybir.ActivationFunctionType.Identity
Ln = mybir.ActivationFunctionType.Ln
DR = mybir.MatmulPerfMode.DoubleRow
NEG_BIG = -1e30

WO_SCALE = 64.0  # host-side Wo prescale before fp8 split
A_SCALE = 32.0   # device-side attention-out prescale (via ones = 1/32)


def _body(tc, xc, wq_d, bq_d, wtc_d, wo_hi_d, wo_lo_d, outT_d):
    nc = tc.nc
    dscr = nc.dram_tensor("dscr", (HPC, N), F32, kind="Internal").ap()

    with (
        tc.tile_pool(name="consts", bufs=1) as consts,
        tc.tile_pool(name="persist", bufs=1) as persist,
    ):
        # ---- constants ----
        ones32 = consts.tile([128, 1], BF16)
        nc.vector.memset(ones32, 1.0 / A_SCALE)
        one_f32 = consts.tile([1, 1], F32)
        nc.vector.memset(one_f32, 1.0)
        ones128_f32 = consts.tile([1, 128], F32)
        nc.vector.memset(ones128_f32, 1.0)
        ones128_bf = consts.tile([1, 128], BF16)
        nc.vector.memset(ones128_bf, 1.0)

        # ---- persistent activations ----
        qT = persist.tile([128, HPC, N], BF16)      # [c', h, n] 2 MB
        kT = persist.tile([128, HPC, N], BF16)      # 2 MB
        vnat = persist.tile([128, HPC, 8, C], BF16)  # [j, h, jt, c] 2 MB
        outT_hi = persist.tile([128, HPC, N], FP8)  # 32*(a - bv) hi split
        outT_lo = persist.tile([128, HPC, N], FP8)
        wo_hi = persist.tile([128, HPC, D], FP8)    # [d, h-chunk, o] 2 MB
        wo_lo = persist.tile([128, HPC, D], FP8)
        mask_col = persist.tile([128, 8], F32)
        scale_col = persist.tile([128, 8], F32)
        lnm_col = persist.tile([128, 8], F32)       # ln(mask) exp bias
        kbv = persist.tile([128, 1], F32)           # (sum(m)-1024) * bv
        ksc32 = persist.tile([1, 1], F32)           # (sum(m)-1024)/32
        kneg32 = persist.tile([1, 1], BF16)         # -(sum(m)-1024)/32
        onesrowN = persist.tile([1, N], BF16)
        nc.vector.memset(onesrowN, 1.0)

        # ============ phase 1: x transpose-in, logits, mask, QKV ===========
        with (
            tc.tile_pool(name="ph1big", bufs=1) as ph1big,
            tc.tile_pool(name="mrows", bufs=1) as mrows,
            tc.tile_pool(name="mm_psum", bufs=2, space="PSUM") as mm_psum,
            tc.tile_pool(name="v_psum", bufs=2, space="PSUM") as v_psum,
        ):
            # packed weight loads first (tiny; scalar queue)
            wqkv_sb = consts.tile([C, 3 * C], BF16)
            nc.scalar.dma_start(out=wqkv_sb, in_=wq_d)
            wq_sb = wqkv_sb[:, 0:C]
            wk_sb = wqkv_sb[:, C : 2 * C]
            wv_sb = wqkv_sb[:, 2 * C : 3 * C]
            bqkv_sb = consts.tile([C, 3], F32)
            nc.scalar.dma_start(out=bqkv_sb, in_=bq_d)
            bq_sb = bqkv_sb[:, 0:1]
            bk_sb = bqkv_sb[:, 1:2]
            bv_sb = bqkv_sb[:, 2:3]
            wtc_sb = consts.tile([C, 1], BF16)
            nc.scalar.dma_start(out=wtc_sb, in_=wtc_d)

            # x transposed in by the DMA XBAR in 8 strided sweeps (pipelined
            # so logits can chase them). xc rows viewed as [(n k), c] with
            # row-stride 256B give layout xT[c, n, k] (k fastest on free).
            NP = 8
            PR = N // NP  # 128 tokens per piece
            xT = ph1big.tile([128, N, NCHUNK], BF16)  # [c, n, k] 4 MB
            xc_rows = xc.rearrange("n (k c) -> (n k) c", c=128)
            tp_insts = []
            for p in range(NP):
                hr = slice(p * PR * NCHUNK, (p + 1) * PR * NCHUNK)
                tp_insts.append(nc.sync.dma_start_transpose(
                    out=xT[:, p * PR : (p + 1) * PR, :],
                    in_=xc_rows[hr, :],
                ))

            # Wo hi/lo splits: one big DMA each, after the x transposes.
            for wo_sb, wo_src, dep in (
                (wo_hi, wo_hi_d, tp_insts[-2]),
                (wo_lo, wo_lo_d, tp_insts[-1]),
            ):
                wi = nc.gpsimd.dma_start(
                    out=wo_sb,
                    in_=wo_src.rearrange("(h p) o -> p h o", p=128),
                )
                add_dep_helper(wi.ins, dep.ins, sync=True, reason="wo after x")

            with tc.tile_pool(name="lg_psum", bufs=1, space="PSUM") as lg_psum:
                # logits[n] = sum_k xT[:, n, k] . wtc   (wtc = (Wl@Wq)/16),
                # one accumulation group per transpose piece so PE starts as
                # soon as the first piece lands.
                lg = lg_psum.tile([1, N], F32)
                negrow = mrows.tile([1, N], F32)
                for p in range(NP):
                    for k in range(NCHUNK):
                        nc.tensor.matmul(
                            lg[:, p * PR : (p + 1) * PR],
                            wtc_sb,
                            xT[:, p * PR : (p + 1) * PR, k],
                            start=(k == 0),
                            stop=(k == NCHUNK - 1),
                        )
                    # negate per piece: runs on DVE while later pieces land
                    nc.vector.tensor_scalar_mul(
                        negrow[:, p * PR : (p + 1) * PR],
                        lg[:, p * PR : (p + 1) * PR], -1.0,
                    )

                # ---- mask: softmax over tokens, snap all but 25 smallest
                # to 1. DVE runs the serial top-k; Act computes the softmax
                # normalization in parallel.
                m8 = mrows.tile([1, 8], F32)
                for _ in range(3):
                    nc.vector.max(out=m8, in_=negrow)
                    nc.vector.match_replace(
                        out=negrow, in_to_replace=m8, in_values=negrow,
                        imm_value=NEG_BIG,
                    )
                nc.vector.max(out=m8, in_=negrow)  # 25th largest of -L
                thrneg = mrows.tile([1, 1], F32)
                nc.vector.tensor_scalar_mul(thrneg, m8[:, 0:1], -1.0)
                urow = mrows.tile([1, N], F32)
                ssum = mrows.tile([1, 1], F32)
                nc.scalar.activation(
                    out=urow, in_=lg, func=Exp, accum_out=ssum
                )
                srecip = mrows.tile([1, 1], F32)
                nc.vector.reciprocal(srecip, ssum)
                # normalize on DVE: on the (in-order) Act queue this would
                # block all phase-1 Q/K moves behind the srecip wait
                smrow = mrows.tile([1, N], F32)
                nc.vector.tensor_scalar_mul(smrow, urow, srecip)
                sel = mrows.tile([1, N], U32)
                nc.vector.tensor_scalar(
                    sel, lg, thrneg, None, op0=mybir.AluOpType.is_gt
                )
                onesrow = mrows.tile([1, N], F32)
                nc.vector.memset(onesrow, 1.0)
                nc.vector.copy_predicated(smrow, sel, onesrow)
                # k = sum(mask) - 1024 (~ -25 + tiny): V-bias fold + dn fixup
                msum = mrows.tile([1, 1], F32)
                mdummy = mrows.tile([1, N], F32)
                nc.vector.tensor_scalar(
                    mdummy, smrow, 1.0, 0.0, op0=mybir.AluOpType.mult,
                    op1=mybir.AluOpType.add, accum_out=msum,
                )
                ksc = mrows.tile([1, 1], F32)
                nc.vector.tensor_scalar_add(ksc, msum, -float(N))
                nc.vector.tensor_scalar_mul(ksc32, ksc, 1.0 / A_SCALE)
                nc.vector.tensor_scalar_mul(kneg32, ksc, -1.0 / A_SCALE)

            # ---- Q/K/V projections, interleaved per head -----------------
            # Q/K produce transposed layouts [c', h, n]; V goes directly to
            # its PV-ready [token, channel] layout (x chunk as stationary).
            # V bias is handled via kbv + host bo fold; the V mask lives in
            # the exp bias (lnm_col), so V copies have no mask dependency.
            # Act paces the Q/K bias-moves; V copies go to DVE (idle once
            # the mask chain drains) except the last heads on Act.
            with tc.tile_pool(name="mc_psum", bufs=1, space="PSUM") as mc_psum:
                for h in range(HPC):
                    for jtg in range(2):
                        vp = v_psum.tile([128, 4, C], F32)
                        for dj in range(4):
                            jt = jtg * 4 + dj
                            nc.tensor.matmul(
                                vp[:, dj, :],
                                xT[:, jt * 128 : (jt + 1) * 128, h],
                                wv_sb,
                                start=True,
                                stop=True,
                            )
                        dst = vnat[:, h, jtg * 4 : (jtg + 1) * 4, :]
                        # all V copies on DVE: the Act queue tail (urow +
                        # 16 Q/K moves) is the phase-2 gate, DVE has slack
                        nc.vector.tensor_copy(dst, vp)
                    for w_sb, b_sb, dstT in ((wq_sb, bq_sb, qT), (wk_sb, bk_sb, kT)):
                        pp = mm_psum.tile([128, N], F32)
                        for half in range(2):
                            nc.tensor.matmul(
                                pp[:, half * 512 : (half + 1) * 512],
                                w_sb,
                                xT[:, half * 512 : (half + 1) * 512, h],
                                start=True,
                                stop=True,
                            )
                        nc.scalar.activation(
                            out=dstT[:, h, :], in_=pp, func=Ident, bias=b_sb
                        )
                    if h == 3:
                        # ---- mask row -> [128, 8] columns via tiny PE
                        # transposes (plus a [128,1] broadcast of k),
                        # replacing two DRAM bounces. Emitted mid-QKV: the
                        # PE is consumer-paced here (idle slots), smrow is
                        # ready by now, and scale_col/lnm_col stop gating
                        # phase 2's first exp.
                        mcol_ps = mc_psum.tile([128, 9], F32)
                        for t in range(8):
                            nc.tensor.transpose(
                                mcol_ps[:, t : t + 1],
                                smrow[:, t * 128 : (t + 1) * 128],
                                one_f32,
                            )
                        nc.tensor.matmul(
                            mcol_ps[:, 8:9], ones128_f32, ksc,
                            start=True, stop=True,
                        )
                        nc.vector.tensor_copy(mask_col, mcol_ps[:, 0:8])
                        nc.vector.tensor_scalar_mul(scale_col, mask_col, SCALE)
                        nc.vector.tensor_mul(kbv, mcol_ps[:, 8:9], bv_sb)
                    if h == 7:
                        # Ln deferred past all Q/K moves: it waits on
                        # mask_col, and anywhere earlier it stalls the
                        # in-order Act queue ahead of the remaining moves
                        nc.scalar.activation(
                            out=lnm_col, in_=mask_col, func=Ln
                        )

        # ================= phase 2: attention ==============================
        # Pool open order places dn/ot on the earliest-freed phase-1 banks.
        with (
            tc.tile_pool(name="pexp", bufs=2) as pexp_pool,
            tc.tile_pool(name="dvp", bufs=2) as dvp,
            tc.tile_pool(name="dn_psum", bufs=1, space="PSUM") as dn_psum,
            tc.tile_pool(name="ot_psum", bufs=1, space="PSUM") as ot_psum,
            tc.tile_pool(name="st_psum", bufs=2, space="PSUM") as st_psum,
        ):
            for h in range(HPC):
                ot = ot_psum.tile([128, N], F32, tag="ot")
                dn = dn_psum.tile([1, N], F32, tag="dn")
                pexp = pexp_pool.tile([128, 8, N], BF16, tag="pexp")

                def emit_pvdn(jt, ot=ot, dn=dn, pexp=pexp, h=h):
                    for half in range(2):
                        sl = slice(half * 512, (half + 1) * 512)
                        nc.tensor.matmul(
                            ot[:, sl],
                            vnat[:, h, jt, :],
                            pexp[:, jt, sl],
                            start=(jt == 0),
                            stop=(jt == 7),
                        )
                        nc.tensor.matmul(
                            dn[:, sl],
                            ones32,
                            pexp[:, jt, sl],
                            start=(jt == 0),
                            stop=(jt == 7 and h < HPC - 1),
                        )

                pending = None
                for jt in range(8):
                    st = st_psum.tile([128, N], F32, tag="st")
                    for half in range(2):
                        nc.tensor.matmul(
                            st[:, half * 512 : (half + 1) * 512],
                            kT[:, h, jt * 128 : (jt + 1) * 128],
                            qT[:, h, half * 512 : (half + 1) * 512],
                            start=True,
                            stop=True,
                        )
                    nc.scalar.activation(
                        out=pexp[:, jt, :], in_=st, func=Exp,
                        scale=scale_col[:, jt : jt + 1],
                        bias=lnm_col[:, jt : jt + 1],
                    )
                    if pending is not None:
                        emit_pvdn(pending)
                    pending = jt
                emit_pvdn(pending)
                if h == HPC - 1:
                    # fold -k/32 into dn in-PSUM (K=1 matmul): removes the
                    # DVE dnadj step from the to_out-gating drain chain
                    for half in range(2):
                        sl = slice(half * 512, (half + 1) * 512)
                        nc.tensor.matmul(
                            dn[:, sl], kneg32, onesrowN[:, sl],
                            start=False, stop=True,
                        )

                # drain: otsb = u + k*bv (DVE per-partition add, frees ot);
                # dn' holds sum_j m_j p_j / 32, true dn/32 = dn' - k/32 (the
                # 25 masked tokens have p ~= 1); tmp = otsb * (32/dn);
                # hi/lo fp8 split of tmp.
                otsb = dvp.tile([128, N], BF16, tag="otsb")
                if h < HPC - 1:
                    nc.vector.tensor_scalar(
                        otsb, ot, kbv, None, op0=mybir.AluOpType.add
                    )
                else:
                    # last head: Act is idle here; keep DVE free for the
                    # to_out-gating dnadj/recip/tmp/hi/lo chain
                    nc.scalar.activation(
                        out=otsb, in_=ot, func=Ident, bias=kbv
                    )
                if h < HPC - 1:
                    dnadj = dvp.tile([1, N], F32, tag="dnadj")
                    nc.vector.tensor_scalar(
                        dnadj, dn, ksc32, None, op0=mybir.AluOpType.subtract
                    )
                    rrow = dvp.tile([1, N], F32, tag="rrow")
                    nc.vector.reciprocal(rrow, dnadj)
                    w_i = nc.sync.dma_start(out=dscr[h, :], in_=rrow)
                    rb = dvp.tile([128, N], F32, tag="rb")
                    r_i = nc.sync.dma_start(
                        out=rb, in_=dscr[h, :].partition_broadcast(128)
                    )
                    add_dep_helper(r_i.ins, w_i.ins, sync=True,
                                   reason="recip RAW")
                else:
                    # last head gates to_out: broadcast 1/dn across partitions
                    # with a K=1 matmul into the freed ot slot instead of the
                    # (slower) DRAM round-trip.
                    rrow_bf = dvp.tile([1, N], BF16, tag="rrowbf")
                    with nc.allow_low_precision(
                        reason="1/dn broadcast row; 0.4% relative is fine"
                    ):
                        nc.vector.reciprocal(rrow_bf, dn)
                    rb = ot_psum.tile([128, N], F32, tag="ot")
                    for half in range(2):
                        sl = slice(half * 512, (half + 1) * 512)
                        nc.tensor.matmul(
                            rb[:, sl], ones128_bf, rrow_bf[:, sl],
                            start=True, stop=True,
                        )
                tmp = dvp.tile([128, N], BF16, tag="tmp")
                nc.vector.tensor_mul(tmp, otsb, rb)
                nc.vector.tensor_copy(outT_hi[:, h, :], tmp)
                nc.vector.tensor_sub(
                    outT_lo[:, h, :], tmp, outT_hi[:, h, :]
                )

            # ============= phase 4: to_out partial (fp8 DoubleRow) =========
            # fo shares the st_psum slots (same shape) so Wo accumulation can
            # begin as soon as the last exp frees an ST slot.
            with tc.tile_pool(name="fout", bufs=3) as fout_pool:
                PRODUCTS = ((wo_hi, outT_hi), (wo_hi, outT_lo), (wo_lo, outT_hi))

                def finish_oc(oc, fo):
                    fout = fout_pool.tile([128, N], BF16)
                    eng = nc.sync if oc % 2 == 0 else nc.gpsimd
                    if oc < 15:
                        nc.vector.tensor_scalar_mul(
                            fout, fo, 1.0 / (WO_SCALE * A_SCALE)
                        )
                        eng.dma_start(
                            out=outT_d[oc * 128 : (oc + 1) * 128, :], in_=fout
                        )
                    else:
                        # final oc: drain per half so the tail DMA covers
                        # only 512 columns
                        for sh in range(2):
                            ssl = slice(sh * 512, (sh + 1) * 512)
                            nc.vector.tensor_scalar_mul(
                                fout[:, ssl], fo[:, ssl],
                                1.0 / (WO_SCALE * A_SCALE),
                            )
                            eng = nc.sync if sh == 0 else nc.gpsimd
                            eng.dma_start(
                                out=outT_d[oc * 128 : (oc + 1) * 128, ssl],
                                in_=fout[:, ssl],
                            )

                pending_oc = None
                for oc in range(16):
                    fo = st_psum.tile([128, N], F32, tag="st")
                    for half in range(2):
                        sl = slice(half * 512, (half + 1) * 512)
                        # t=3 (heads 6,7) last in every product so the first
                        # 9 instructions don't wait on head 7's drain chain
                        steps = [(w, a, t) for t in (0, 1, 2)
                                 for (w, a) in PRODUCTS]
                        steps += [(w, a, 3) for (w, a) in PRODUCTS]
                        for i, (wsp, asp, t) in enumerate(steps):
                            nc.tensor.matmul(
                                fo[:, sl],
                                wsp[:, 2 * t : 2 * t + 2,
                                    oc * 128 : (oc + 1) * 128],
                                asp[:, 2 * t : 2 * t + 2, sl],
                                start=(i == 0),
                                stop=(i == len(steps) - 1),
                                perf_mode=DR,
                            )
                    if pending_oc is not None:
                        finish_oc(*pending_oc)
                    pending_oc = (oc, fo)
                finish_oc(*pending_oc)


_CACHE = {}


def _get_module():
    if "nc" in _CACHE:
        return _CACHE["nc"]
    nc = bacc.Bacc("TRN2", target_bir_lowering=False, debug=False, num_devices=8)
    xc = nc.dram_tensor("xc", (N, D), BF16, kind="ExternalInput").ap()
    wq_d = nc.dram_tensor("wqkvT", (C, 3 * C), BF16, kind="ExternalInput").ap()
    bq_d = nc.dram_tensor("bqkv", (C, 3), F32, kind="ExternalInput").ap()
    wtc_d = nc.dram_tensor("wtc", (C, 1), BF16, kind="ExternalInput").ap()
    wo_hi_d = nc.dram_tensor("woHi", (HPC * C, D), FP8, kind="ExternalInput").ap()
    wo_lo_d = nc.dram_tensor("woLo", (HPC * C, D), FP8, kind="ExternalInput").ap()
    outT_d = nc.dram_tensor("outT", (D, N), BF16, kind="ExternalOutput").ap()

    with tile.TileContext(nc) as tc:
        _body(tc, xc, wq_d, bq_d, wtc_d, wo_hi_d, wo_lo_d, outT_d)
    nc.compile()
    _CACHE["nc"] = nc
    return nc


def make_in_maps(x, Wq, bq, Wk, bk, Wv, bv, Wl, bl, Wo, bo):
    bf16 = ml_dtypes.bfloat16
    fp8 = ml_dtypes.float8_e4m3
    x = np.ascontiguousarray(np.asarray(x, np.float32))
    Wq = np.asarray(Wq, np.float32)
    Wk = np.asarray(Wk, np.float32)
    Wv = np.asarray(Wv, np.float32)
    Wl = np.asarray(Wl, np.float32)
    Wo = np.asarray(Wo, np.float32)
    we = (Wl[0] @ Wq) / float(NCHUNK)  # (128,) logits weight per chunk
    common = {
        "wqkvT": np.ascontiguousarray(
            np.concatenate([Wq.T, Wk.T, Wv.T], axis=1)
        ).astype(bf16),
        "bqkv": np.ascontiguousarray(np.stack(
            [np.asarray(bq, np.float32), np.asarray(bk, np.float32),
             np.asarray(bv, np.float32)], axis=1
        )),
        "wtc": we.astype(bf16).reshape(C, 1),
    }
    woT = np.ascontiguousarray(Wo.T) * WO_SCALE  # (d, o), prescaled
    in_maps = []
    halves = {}
    for g in range(2):
        wh = woT[g * 1024 : (g + 1) * 1024, :]
        hi = wh.astype(fp8)
        lo = (wh - hi.astype(np.float32)).astype(fp8)
        halves[g] = (np.ascontiguousarray(hi), np.ascontiguousarray(lo))
    for core in range(8):
        b, g = divmod(core, 2)
        xb = x[b]
        xcore = xb if g == 0 else np.concatenate(
            [xb[:, 1024:], xb[:, :1024]], axis=1
        )
        in_maps.append({
            "xc": np.ascontiguousarray(xcore.astype(bf16)),
            "woHi": halves[g][0],
            "woLo": halves[g][1],
            **common,
        })
    return in_maps


def run_spmd(in_maps, trace=False, **kw):
    nc = _get_module()
    return bass_utils.run_bass_kernel_spmd(
        nc, in_maps, core_ids=list(range(8)), trace=trace, **kw
    )


def gather(results, Wo, bv, bo):
    Wo = np.asarray(Wo, np.float32)
    bv = np.asarray(bv, np.float32)
    bo = np.asarray(bo, np.float32)
    # a = a_tilde + bv per head-channel: fold bv through Wo into the bias.
    bo_eff = bo + np.tile(bv, NCHUNK) @ Wo.T
    out = np.empty((B, N, D), np.float32)
    for b in range(B):
        p0 = results[2 * b]["outT"].astype(np.float32).T
        p1 = results[2 * b + 1]["outT"].astype(np.float32).T
        out[b] = p0 + p1 + bo_eff
    return out


def kernel(x, Wq, bq, Wk, bk, Wv, bv, Wl, bl, Wo, bo, stage=None, **_unused):
    in_maps = make_in_maps(x, Wq, bq, Wk, bk, Wv, bv, Wl, bl, Wo, bo)
    try:
        res = run_spmd(in_maps)
    except Exception:
        # transient device/runtime hiccup: retry once after a short pause
        import time as _time

        _time.sleep(2.0)
        res = run_spmd(in_maps)
    return gather(res.results, Wo, bv, bo)


# revision 54
# speedup vs baseline: 1.0763x; 1.0050x over previous
"""Trainium2 Bass kernel for nn_Attention_54614804136573 (topk_masking).

Sharding: 8 cores = 4 batches x 2 head-groups (8 heads each). Each core gets
its batch's full x (columns rotated so its own 8 head-chunks come first),
computes the token-importance mask redundantly, runs its 8 heads of attention,
and produces a partial to_out product over its 1024-wide d-slice for all 2048
output channels. The host sums the two partials per batch and adds bo'
(bo with the V-bias term folded in).

Key structure vs the previous version:
- x arrives as bf16 and is transposed into SBUF chunk-wise by the DMA XBAR
  (dma_start_transpose), removing all PE transposes and PSUM copies for x.
- V is projected directly into its PV-ready [token, channel] layout by using
  the x chunk as the stationary operand (out = xT_chunk.T @ WvT), removing
  the separate V transpose pass. The V bias is exactly handled outside the
  kernel: a = u/dn + bv*(S_pm/dn) with S_pm ~= dn + (sum(mask)-1024), so the
  per-head bias folds into an Act bias (k*bv) plus a host-side bo term.
- The softmax denominator is an M=1 ones-matmul accumulated over all 8
  j-tiles into a [1, N] PSUM row (ones = 1/32 so the normalized output is
  pre-scaled by 32 for fp8).
- to_out runs in fp8 DoubleRow (K=256/instr at 0.5 cycles/row) with an
  error-compensated hi+lo split of both Wo (host side, x64) and the
  attention output (device side, x32): w*a ~= w_hi*a_hi + w_hi*a_lo +
  w_lo*a_hi, 12 DR matmuls per (oc, half) instead of 16 bf16 matmuls.
"""

import sys

sys.path.insert(0, "/opt/trn_rl_repo")

import numpy as np
import ml_dtypes

import concourse.mybir as mybir
import concourse.tile as tile
from concourse import bacc, bass_utils
from concourse.tile import add_dep_helper

B = 4
N = 1024
C = 128
D = 2048
NCHUNK = 16  # d-chunks of 128 (= patch positions = heads)
HPC = 8  # heads per core
MASK_NUM = 25
SCALE = 64.0 ** -0.5  # 0.125

F32 = mybir.dt.float32
BF16 = mybir.dt.bfloat16
FP8 = mybir.dt.float8e4
U32 = mybir.dt.uint32
Exp = mybir.ActivationFunctionType.Exp
Ident = mybir.ActivationFunctionType.Identity
Ln = mybir.ActivationFunctionType.Ln
DR = mybir.MatmulPerfMode.DoubleRow
NEG_BIG = -1e30

WO_SCALE = 64.0  # host-side Wo prescale before fp8 split
A_SCALE = 32.0   # device-side attention-out prescale (via ones = 1/32)


def _body(tc, xc, wq_d, bq_d, wtc_d, wo_hi_d, wo_lo_d, outT_d):
    nc = tc.nc
    dscr = nc.dram_tensor("dscr", (HPC, N), F32, kind="Internal").ap()

    with (
        tc.tile_pool(name="consts", bufs=1) as consts,
        tc.tile_pool(name="persist", bufs=1) as persist,
    ):
        # ---- constants ----
        ones32 = consts.tile([128, 1], BF16)
        nc.vector.memset(ones32, 1.0 / A_SCALE)
        one_f32 = consts.tile([1, 1], F32)
        nc.vector.memset(one_f32, 1.0)
        ones128_f32 = consts.tile([1, 128], F32)
        nc.vector.memset(ones128_f32, 1.0)
        ones128_bf = consts.tile([1, 128], BF16)
        nc.vector.memset(ones128_bf, 1.0)

        # ---- persistent activations ----
        qT = persist.tile([128, HPC, N], BF16)      # [c', h, n] 2 MB
        kT = persist.tile([128, HPC, N], BF16)      # 2 MB
        vnat = persist.tile([128, HPC, 8, C], BF16)  # [j, h, jt, c] 2 MB
        outT_hi = persist.tile([128, HPC, N], FP8)  # 32*(a - bv) hi split
        outT_lo = persist.tile([128, HPC, N], FP8)
        wo_hi = persist.tile([128, HPC, D], FP8)    # [d, h-chunk, o] 2 MB
        wo_lo = persist.tile([128, HPC, D], FP8)
        mask_col = persist.tile([128, 8], F32)
        scale_col = persist.tile([128, 8], F32)
        lnm_col = persist.tile([128, 8], F32)       # ln(mask) exp bias
        kbv = persist.tile([128, 1], F32)           # (sum(m)-1024) * bv
        ksc32 = persist.tile([1, 1], F32)           # (sum(m)-1024)/32
        kneg32 = persist.tile([1, 1], BF16)         # -(sum(m)-1024)/32
        onesrowN = persist.tile([1, N], BF16)
        nc.vector.memset(onesrowN, 1.0)

        # ============ phase 1: x transpose-in, logits, mask, QKV ===========
        with (
            tc.tile_pool(name="ph1big", bufs=1) as ph1big,
            tc.tile_pool(name="mrows", bufs=1) as mrows,
            tc.tile_pool(name="mm_psum", bufs=2, space="PSUM") as mm_psum,
            tc.tile_pool(name="v_psum", bufs=2, space="PSUM") as v_psum,
        ):
            # packed weight loads first (tiny; scalar queue)
            wqkv_sb = consts.tile([C, 3 * C], BF16)
            nc.scalar.dma_start(out=wqkv_sb, in_=wq_d)
            wq_sb = wqkv_sb[:, 0:C]
            wk_sb = wqkv_sb[:, C : 2 * C]
            wv_sb = wqkv_sb[:, 2 * C : 3 * C]
            bqkv_sb = consts.tile([C, 3], F32)
            nc.scalar.dma_start(out=bqkv_sb, in_=bq_d)
            bq_sb = bqkv_sb[:, 0:1]
            bk_sb = bqkv_sb[:, 1:2]
            bv_sb = bqkv_sb[:, 2:3]
            wtc_sb = consts.tile([C, 1], BF16)
            nc.scalar.dma_start(out=wtc_sb, in_=wtc_d)

            # x transposed in by the DMA XBAR in 8 strided sweeps (pipelined
            # so logits can chase them). xc rows viewed as [(n k), c] with
            # row-stride 256B give layout xT[c, n, k] (k fastest on free).
            NP = 8
            PR = N // NP  # 128 tokens per piece
            xT = ph1big.tile([128, N, NCHUNK], BF16)  # [c, n, k] 4 MB
            xc_rows = xc.rearrange("n (k c) -> (n k) c", c=128)
            tp_insts = []
            for p in range(NP):
                hr = slice(p * PR * NCHUNK, (p + 1) * PR * NCHUNK)
                tp_insts.append(nc.sync.dma_start_transpose(
                    out=xT[:, p * PR : (p + 1) * PR, :],
                    in_=xc_rows[hr, :],
                ))

            # Wo hi/lo splits: one big DMA each, after the x transposes.
            for wo_sb, wo_src, dep in (
                (wo_hi, wo_hi_d, tp_insts[-2]),
                (wo_lo, wo_lo_d, tp_insts[-1]),
            ):
                wi = nc.gpsimd.dma_start(
                    out=wo_sb,
                    in_=wo_src.rearrange("(h p) o -> p h o", p=128),
                )
                add_dep_helper(wi.ins, dep.ins, sync=True, reason="wo after x")

            with tc.tile_pool(name="lg_psum", bufs=1, space="PSUM") as lg_psum:
                # logits[n] = sum_k xT[:, n, k] . wtc   (wtc = (Wl@Wq)/16),
                # one accumulation group per transpose piece so PE starts as
                # soon as the first piece lands.
                lg = lg_psum.tile([1, N], F32)
                negrow = mrows.tile([1, N], F32)
                for p in range(NP):
                    for k in range(NCHUNK):
                        nc.tensor.matmul(
                            lg[:, p * PR : (p + 1) * PR],
                            wtc_sb,
                            xT[:, p * PR : (p + 1) * PR, k],
                            start=(k == 0),
                            stop=(k == NCHUNK - 1),
                        )
                    # negate per piece: runs on DVE while later pieces land
                    nc.vector.tensor_scalar_mul(
                        negrow[:, p * PR : (p + 1) * PR],
                        lg[:, p * PR : (p + 1) * PR], -1.0,
                    )

                # ---- mask: softmax over tokens, snap all but 25 smallest
                # to 1. DVE runs the serial top-k; Act computes the softmax
                # normalization in parallel.
                m8 = mrows.tile([1, 8], F32)
                for _ in range(3):
                    nc.vector.max(out=m8, in_=negrow)
                    nc.vector.match_replace(
                        out=negrow, in_to_replace=m8, in_values=negrow,
                        imm_value=NEG_BIG,
                    )
                nc.vector.max(out=m8, in_=negrow)  # 25th largest of -L
                thrneg = mrows.tile([1, 1], F32)
                nc.vector.tensor_scalar_mul(thrneg, m8[:, 0:1], -1.0)
                urow = mrows.tile([1, N], F32)
                ssum = mrows.tile([1, 1], F32)
                nc.scalar.activation(
                    out=urow, in_=lg, func=Exp, accum_out=ssum
                )
                srecip = mrows.tile([1, 1], F32)
                nc.vector.reciprocal(srecip, ssum)
                # normalize on DVE: on the (in-order) Act queue this would
                # block all phase-1 Q/K moves behind the srecip wait
                smrow = mrows.tile([1, N], F32)
                nc.vector.tensor_scalar_mul(smrow, urow, srecip)
                sel = mrows.tile([1, N], U32)
                nc.vector.tensor_scalar(
                    sel, lg, thrneg, None, op0=mybir.AluOpType.is_gt
                )
                onesrow = mrows.tile([1, N], F32)
                nc.vector.memset(onesrow, 1.0)
                nc.vector.copy_predicated(smrow, sel, onesrow)
                # k = sum(mask) - 1024 (~ -25 + tiny): V-bias fold + dn fixup
                msum = mrows.tile([1, 1], F32)
                mdummy = mrows.tile([1, N], F32)
                nc.vector.tensor_scalar(
                    mdummy, smrow, 1.0, 0.0, op0=mybir.AluOpType.mult,
                    op1=mybir.AluOpType.add, accum_out=msum,
                )
                ksc = mrows.tile([1, 1], F32)
                nc.vector.tensor_scalar_add(ksc, msum, -float(N))
                nc.vector.tensor_scalar_mul(ksc32, ksc, 1.0 / A_SCALE)
                nc.vector.tensor_scalar_mul(kneg32, ksc, -1.0 / A_SCALE)

            # ---- Q/K/V projections, interleaved per head -----------------
            # Q/K produce transposed layouts [c', h, n]; V goes directly to
            # its PV-ready [token, channel] layout (x chunk as stationary).
            # V bias is handled via kbv + host bo fold; the V mask lives in
            # the exp bias (lnm_col), so V copies have no mask dependency.
            # Act paces the Q/K bias-moves; V copies go to DVE (idle once
            # the mask chain drains) except the last heads on Act.
            with tc.tile_pool(name="mc_psum", bufs=1, space="PSUM") as mc_psum:
                for h in range(HPC):
                    for jtg in range(2):
                        vp = v_psum.tile([128, 4, C], F32)
                        for dj in range(4):
                            jt = jtg * 4 + dj
                            nc.tensor.matmul(
                                vp[:, dj, :],
                                xT[:, jt * 128 : (jt + 1) * 128, h],
                                wv_sb,
                                start=True,
                                stop=True,
                            )
                        dst = vnat[:, h, jtg * 4 : (jtg + 1) * 4, :]
                        # all V copies on DVE: the Act queue tail (urow +
                        # 16 Q/K moves) is the phase-2 gate, DVE has slack
                        nc.vector.tensor_copy(dst, vp)
                    for w_sb, b_sb, dstT in ((wq_sb, bq_sb, qT), (wk_sb, bk_sb, kT)):
                        pp = mm_psum.tile([128, N], F32)
                        for half in range(2):
                            nc.tensor.matmul(
                                pp[:, half * 512 : (half + 1) * 512],
                                w_sb,
                                xT[:, half * 512 : (half + 1) * 512, h],
                                start=True,
                                stop=True,
                            )
                        nc.scalar.activation(
                            out=dstT[:, h, :], in_=pp, func=Ident, bias=b_sb
                        )
                    if h == 3:
                        # ---- mask row -> [128, 8] columns via tiny PE
                        # transposes (plus a [128,1] broadcast of k),
                        # replacing two DRAM bounces. Emitted mid-QKV: the
                        # PE is consumer-paced here (idle slots), smrow is
                        # ready by now, and scale_col/lnm_col stop gating
                        # phase 2's first exp.
                        mcol_ps = mc_psum.tile([128, 9], F32)
                        for t in range(8):
                            nc.tensor.transpose(
                                mcol_ps[:, t : t + 1],
                                smrow[:, t * 128 : (t + 1) * 128],
                                one_f32,
                            )
                        nc.tensor.matmul(
                            mcol_ps[:, 8:9], ones128_f32, ksc,
                            start=True, stop=True,
                        )
                        nc.vector.tensor_copy(mask_col, mcol_ps[:, 0:8])
                        nc.vector.tensor_scalar_mul(scale_col, mask_col, SCALE)
                        nc.vector.tensor_mul(kbv, mcol_ps[:, 8:9], bv_sb)
                    if h == 7:
                        # Ln deferred past all Q/K moves: it waits on
                        # mask_col, and anywhere earlier it stalls the
                        # in-order Act queue ahead of the remaining moves
                        nc.scalar.activation(
                            out=lnm_col, in_=mask_col, func=Ln
                        )

        # ================= phase 2: attention ==============================
        # Pool open order places dn/ot on the earliest-freed phase-1 banks.
        with (
            tc.tile_pool(name="pexp", bufs=2) as pexp_pool,
            tc.tile_pool(name="dvp", bufs=2) as dvp,
            tc.tile_pool(name="dn_psum", bufs=1, space="PSUM") as dn_psum,
            tc.tile_pool(name="ot_psum", bufs=1, space="PSUM") as ot_psum,
            tc.tile_pool(name="st_psum", bufs=2, space="PSUM") as st_psum,
        ):
            for h in range(HPC):
                ot = ot_psum.tile([128, N], F32, tag="ot")
                dn = dn_psum.tile([1, N], F32, tag="dn")
                pexp = pexp_pool.tile([128, 8, N], BF16, tag="pexp")

                def emit_pvdn(jt, ot=ot, dn=dn, pexp=pexp, h=h):
                    for half in range(2):
                        sl = slice(half * 512, (half + 1) * 512)
                        nc.tensor.matmul(
                            ot[:, sl],
                            vnat[:, h, jt, :],
                            pexp[:, jt, sl],
                            start=(jt == 0),
                            stop=(jt == 7),
                        )
                        nc.tensor.matmul(
                            dn[:, sl],
                            ones32,
                            pexp[:, jt, sl],
                            start=(jt == 0),
                            stop=(jt == 7 and h < HPC - 1),
                        )

                pending = None
                for jt in range(8):
                    st = st_psum.tile([128, N], F32, tag="st")
                    for half in range(2):
                        nc.tensor.matmul(
                            st[:, half * 512 : (half + 1) * 512],
                            kT[:, h, jt * 128 : (jt + 1) * 128],
                            qT[:, h, half * 512 : (half + 1) * 512],
                            start=True,
                            stop=True,
                        )
                    nc.scalar.activation(
                        out=pexp[:, jt, :], in_=st, func=Exp,
                        scale=scale_col[:, jt : jt + 1],
                        bias=lnm_col[:, jt : jt + 1],
                    )
                    if pending is not None:
                        emit_pvdn(pending)
                    pending = jt
                emit_pvdn(pending)
                if h == HPC - 1:
                    # fold -k/32 into dn in-PSUM (K=1 matmul): removes the
                    # DVE dnadj step from the to_out-gating drain chain
                    for half in range(2):
                        sl = slice(half * 512, (half + 1) * 512)
                        nc.tensor.matmul(
                            dn[:, sl], kneg32, onesrowN[:, sl],
                            start=False, stop=True,
                        )

                # drain: otsb = u + k*bv (DVE per-partition add, frees ot);
                # dn' holds sum_j m_j p_j / 32, true dn/32 = dn' - k/32 (the
                # 25 masked tokens have p ~= 1); tmp = otsb * (32/dn);
                # hi/lo fp8 split of tmp.
                otsb = dvp.tile([128, N], BF16, tag="otsb")
                if h < HPC - 1:
                    nc.vector.tensor_scalar(
                        otsb, ot, kbv, None, op0=mybir.AluOpType.add
                    )
                else:
                    # last head: Act is idle here; keep DVE free for the
                    # to_out-gating dnadj/recip/tmp/hi/lo chain
                    nc.scalar.activation(
                        out=otsb, in_=ot, func=Ident, bias=kbv
                    )
                if h < HPC - 1:
                    dnadj = dvp.tile([1, N], F32, tag="dnadj")
                    nc.vector.tensor_scalar(
                        dnadj, dn, ksc32, None, op0=mybir.AluOpType.subtract
                    )
                    rrow = dvp.tile([1, N], F32, tag="rrow")
                    nc.vector.reciprocal(rrow, dnadj)
                    w_i = nc.sync.dma_start(out=dscr[h, :], in_=rrow)
                    rb = dvp.tile([128, N], F32, tag="rb")
                    r_i = nc.sync.dma_start(
                        out=rb, in_=dscr[h, :].partition_broadcast(128)
                    )
                    add_dep_helper(r_i.ins, w_i.ins, sync=True,
                                   reason="recip RAW")
                    tmp = dvp.tile([128, N], BF16, tag="tmp")
                    nc.vector.tensor_mul(tmp, otsb, rb)
                    nc.vector.tensor_copy(outT_hi[:, h, :], tmp)
                    nc.vector.tensor_sub(
                        outT_lo[:, h, :], tmp, outT_hi[:, h, :]
                    )
                else:
                    # last head gates to_out: broadcast 1/dn across partitions
                    # with a K=1 matmul into the freed ot slot instead of the
                    # (slower) DRAM round-trip, and run the whole drain per
                    # half-column — the to_out DR matmuls consume hi/lo per
                    # half, so half 0's t3 instructions start ~2us earlier.
                    rrow_bf = dvp.tile([1, N], BF16, tag="rrowbf")
                    rb = ot_psum.tile([128, N], F32, tag="ot")
                    tmp = dvp.tile([128, N], BF16, tag="tmp")
                    for half in range(2):
                        sl = slice(half * 512, (half + 1) * 512)
                        with nc.allow_low_precision(
                            reason="1/dn broadcast row; 0.4% relative is fine"
                        ):
                            nc.vector.reciprocal(rrow_bf[:, sl], dn[:, sl])
                        nc.tensor.matmul(
                            rb[:, sl], ones128_bf, rrow_bf[:, sl],
                            start=True, stop=True,
                        )
                        nc.vector.tensor_mul(
                            tmp[:, sl], otsb[:, sl], rb[:, sl]
                        )
                        nc.vector.tensor_copy(
                            outT_hi[:, h, sl], tmp[:, sl]
                        )
                        nc.vector.tensor_sub(
                            outT_lo[:, h, sl], tmp[:, sl],
                            outT_hi[:, h, sl],
                        )

            # ============= phase 4: to_out partial (fp8 DoubleRow) =========
            # fo shares the st_psum slots (same shape) so Wo accumulation can
            # begin as soon as the last exp frees an ST slot.
            with tc.tile_pool(name="fout", bufs=3) as fout_pool:
                PRODUCTS = ((wo_hi, outT_hi), (wo_hi, outT_lo), (wo_lo, outT_hi))

                def finish_oc(oc, fo):
                    fout = fout_pool.tile([128, N], BF16)
                    eng = nc.sync if oc % 2 == 0 else nc.gpsimd
                    if oc < 15:
                        nc.vector.tensor_scalar_mul(
                            fout, fo, 1.0 / (WO_SCALE * A_SCALE)
                        )
                        eng.dma_start(
                            out=outT_d[oc * 128 : (oc + 1) * 128, :], in_=fout
                        )
                    else:
                        # final oc: drain per half so the tail DMA covers
                        # only 512 columns
                        for sh in range(2):
                            ssl = slice(sh * 512, (sh + 1) * 512)
                            nc.vector.tensor_scalar_mul(
                                fout[:, ssl], fo[:, ssl],
                                1.0 / (WO_SCALE * A_SCALE),
                            )
                            eng = nc.sync if sh == 0 else nc.gpsimd
                            eng.dma_start(
                                out=outT_d[oc * 128 : (oc + 1) * 128, ssl],
                                in_=fout[:, ssl],
                            )

                pending_oc = None
                for oc in range(16):
                    fo = st_psum.tile([128, N], F32, tag="st")
                    for half in range(2):
                        sl = slice(half * 512, (half + 1) * 512)
                        # t=3 (heads 6,7) last in every product so the first
                        # 9 instructions don't wait on head 7's drain chain
                        steps = [(w, a, t) for t in (0, 1, 2)
                                 for (w, a) in PRODUCTS]
                        steps += [(w, a, 3) for (w, a) in PRODUCTS]
                        for i, (wsp, asp, t) in enumerate(steps):
                            nc.tensor.matmul(
                                fo[:, sl],
                                wsp[:, 2 * t : 2 * t + 2,
                                    oc * 128 : (oc + 1) * 128],
                                asp[:, 2 * t : 2 * t + 2, sl],
                                start=(i == 0),
                                stop=(i == len(steps) - 1),
                                perf_mode=DR,
                            )
                    if pending_oc is not None:
                        finish_oc(*pending_oc)
                    pending_oc = (oc, fo)
                finish_oc(*pending_oc)


_CACHE = {}


def _get_module():
    if "nc" in _CACHE:
        return _CACHE["nc"]
    nc = bacc.Bacc("TRN2", target_bir_lowering=False, debug=False, num_devices=8)
    xc = nc.dram_tensor("xc", (N, D), BF16, kind="ExternalInput").ap()
    wq_d = nc.dram_tensor("wqkvT", (C, 3 * C), BF16, kind="ExternalInput").ap()
    bq_d = nc.dram_tensor("bqkv", (C, 3), F32, kind="ExternalInput").ap()
    wtc_d = nc.dram_tensor("wtc", (C, 1), BF16, kind="ExternalInput").ap()
    wo_hi_d = nc.dram_tensor("woHi", (HPC * C, D), FP8, kind="ExternalInput").ap()
    wo_lo_d = nc.dram_tensor("woLo", (HPC * C, D), FP8, kind="ExternalInput").ap()
    outT_d = nc.dram_tensor("outT", (D, N), BF16, kind="ExternalOutput").ap()

    with tile.TileContext(nc) as tc:
        _body(tc, xc, wq_d, bq_d, wtc_d, wo_hi_d, wo_lo_d, outT_d)
    nc.compile()
    _CACHE["nc"] = nc
    return nc


def make_in_maps(x, Wq, bq, Wk, bk, Wv, bv, Wl, bl, Wo, bo):
    bf16 = ml_dtypes.bfloat16
    fp8 = ml_dtypes.float8_e4m3
    x = np.ascontiguousarray(np.asarray(x, np.float32))
    Wq = np.asarray(Wq, np.float32)
    Wk = np.asarray(Wk, np.float32)
    Wv = np.asarray(Wv, np.float32)
    Wl = np.asarray(Wl, np.float32)
    Wo = np.asarray(Wo, np.float32)
    we = (Wl[0] @ Wq) / float(NCHUNK)  # (128,) logits weight per chunk
    common = {
        "wqkvT": np.ascontiguousarray(
            np.concatenate([Wq.T, Wk.T, Wv.T], axis=1)
        ).astype(bf16),
        "bqkv": np.ascontiguousarray(np.stack(
            [np.asarray(bq, np.float32), np.asarray(bk, np.float32),
             np.asarray(bv, np.float32)], axis=1
        )),
        "wtc": we.astype(bf16).reshape(C, 1),
    }
    woT = np.ascontiguousarray(Wo.T) * WO_SCALE  # (d, o), prescaled
    in_maps = []
    halves = {}
    for g in range(2):
        wh = woT[g * 1024 : (g + 1) * 1024, :]
        hi = wh.astype(fp8)
        lo = (wh - hi.astype(np.float32)).astype(fp8)
        halves[g] = (np.ascontiguousarray(hi), np.ascontiguousarray(lo))
    for core in range(8):
        b, g = divmod(core, 2)
        xb = x[b]
        xcore = xb if g == 0 else np.concatenate(
            [xb[:, 1024:], xb[:, :1024]], axis=1
        )
        in_maps.append({
            "xc": np.ascontiguousarray(xcore.astype(bf16)),
            "woHi": halves[g][0],
            "woLo": halves[g][1],
            **common,
        })
    return in_maps


def run_spmd(in_maps, trace=False, **kw):
    nc = _get_module()
    return bass_utils.run_bass_kernel_spmd(
        nc, in_maps, core_ids=list(range(8)), trace=trace, **kw
    )


def gather(results, Wo, bv, bo):
    Wo = np.asarray(Wo, np.float32)
    bv = np.asarray(bv, np.float32)
    bo = np.asarray(bo, np.float32)
    # a = a_tilde + bv per head-channel: fold bv through Wo into the bias.
    bo_eff = bo + np.tile(bv, NCHUNK) @ Wo.T
    out = np.empty((B, N, D), np.float32)
    for b in range(B):
        p0 = results[2 * b]["outT"].astype(np.float32).T
        p1 = results[2 * b + 1]["outT"].astype(np.float32).T
        out[b] = p0 + p1 + bo_eff
    return out


def kernel(x, Wq, bq, Wk, bk, Wv, bv, Wl, bl, Wo, bo, stage=None, **_unused):
    in_maps = make_in_maps(x, Wq, bq, Wk, bk, Wv, bv, Wl, bl, Wo, bo)
    try:
        res = run_spmd(in_maps)
    except Exception:
        # transient device/runtime hiccup: retry once after a short pause
        import time as _time

        _time.sleep(2.0)
        res = run_spmd(in_maps)
    return gather(res.results, Wo, bv, bo)


# revision 55
# speedup vs baseline: 1.0832x; 1.0064x over previous
"""Trainium2 Bass kernel for nn_Attention_54614804136573 (topk_masking).

Sharding: 8 cores = 4 batches x 2 head-groups (8 heads each). Each core gets
its batch's full x (columns rotated so its own 8 head-chunks come first),
computes the token-importance mask redundantly, runs its 8 heads of attention,
and produces a partial to_out product over its 1024-wide d-slice for all 2048
output channels. The host sums the two partials per batch and adds bo'
(bo with the V-bias term folded in).

Key structure vs the previous version:
- x arrives as bf16 and is transposed into SBUF chunk-wise by the DMA XBAR
  (dma_start_transpose), removing all PE transposes and PSUM copies for x.
- V is projected directly into its PV-ready [token, channel] layout by using
  the x chunk as the stationary operand (out = xT_chunk.T @ WvT), removing
  the separate V transpose pass. The V bias is exactly handled outside the
  kernel: a = u/dn + bv*(S_pm/dn) with S_pm ~= dn + (sum(mask)-1024), so the
  per-head bias folds into an Act bias (k*bv) plus a host-side bo term.
- The softmax denominator is an M=1 ones-matmul accumulated over all 8
  j-tiles into a [1, N] PSUM row (ones = 1/32 so the normalized output is
  pre-scaled by 32 for fp8).
- to_out runs in fp8 DoubleRow (K=256/instr at 0.5 cycles/row) with an
  error-compensated hi+lo split of both Wo (host side, x64) and the
  attention output (device side, x32): w*a ~= w_hi*a_hi + w_hi*a_lo +
  w_lo*a_hi, 12 DR matmuls per (oc, half) instead of 16 bf16 matmuls.
"""

import sys

sys.path.insert(0, "/opt/trn_rl_repo")

import numpy as np
import ml_dtypes

import concourse.mybir as mybir
import concourse.tile as tile
from concourse import bacc, bass_utils
from concourse.tile import add_dep_helper

B = 4
N = 1024
C = 128
D = 2048
NCHUNK = 16  # d-chunks of 128 (= patch positions = heads)
HPC = 8  # heads per core
MASK_NUM = 25
SCALE = 64.0 ** -0.5  # 0.125

F32 = mybir.dt.float32
BF16 = mybir.dt.bfloat16
FP8 = mybir.dt.float8e4
U32 = mybir.dt.uint32
Exp = mybir.ActivationFunctionType.Exp
Ident = mybir.ActivationFunctionType.Identity
Ln = mybir.ActivationFunctionType.Ln
DR = mybir.MatmulPerfMode.DoubleRow
NEG_BIG = -1e30

WO_SCALE = 64.0  # host-side Wo prescale before fp8 split
A_SCALE = 32.0   # device-side attention-out prescale (via ones = 1/32)


def _body(tc, xc, wq_d, bq_d, wtc_d, wo_hi_d, wo_lo_d, outT_d):
    nc = tc.nc
    dscr = nc.dram_tensor("dscr", (HPC, N), F32, kind="Internal").ap()

    with (
        tc.tile_pool(name="consts", bufs=1) as consts,
        tc.tile_pool(name="persist", bufs=1) as persist,
    ):
        # ---- constants ----
        ones32 = consts.tile([128, 1], BF16)
        nc.vector.memset(ones32, 1.0 / A_SCALE)
        one_f32 = consts.tile([1, 1], F32)
        nc.vector.memset(one_f32, 1.0)
        ones128_f32 = consts.tile([1, 128], F32)
        nc.vector.memset(ones128_f32, 1.0)
        ones128_bf = consts.tile([1, 128], BF16)
        nc.vector.memset(ones128_bf, 1.0)

        # ---- persistent activations ----
        qT = persist.tile([128, HPC, N], BF16)      # [c', h, n] 2 MB
        kT = persist.tile([128, HPC, N], BF16)      # 2 MB
        vnat = persist.tile([128, HPC, 8, C], BF16)  # [j, h, jt, c] 2 MB
        outT_hi = persist.tile([128, HPC, N], FP8)  # 32*(a - bv) hi split
        outT_lo = persist.tile([128, HPC, N], FP8)
        wo_hi = persist.tile([128, HPC, D], FP8)    # [d, h-chunk, o] 2 MB
        wo_lo = persist.tile([128, HPC, D], FP8)
        mask_col = persist.tile([128, 8], F32)
        scale_col = persist.tile([128, 8], F32)
        lnm_col = persist.tile([128, 8], F32)       # ln(mask) exp bias
        kbv = persist.tile([128, 1], F32)           # (sum(m)-1024) * bv
        ksc32 = persist.tile([1, 1], F32)           # (sum(m)-1024)/32
        kneg32 = persist.tile([1, 1], BF16)         # -(sum(m)-1024)/32
        onesrowN = persist.tile([1, N], BF16)
        nc.vector.memset(onesrowN, 1.0)

        # ============ phase 1: x transpose-in, logits, mask, QKV ===========
        with (
            tc.tile_pool(name="ph1big", bufs=1) as ph1big,
            tc.tile_pool(name="mrows", bufs=1) as mrows,
            tc.tile_pool(name="mm_psum", bufs=2, space="PSUM") as mm_psum,
            tc.tile_pool(name="v_psum", bufs=2, space="PSUM") as v_psum,
        ):
            # packed weight loads first (tiny; scalar queue)
            wqkv_sb = consts.tile([C, 3 * C], BF16)
            nc.scalar.dma_start(out=wqkv_sb, in_=wq_d)
            wq_sb = wqkv_sb[:, 0:C]
            wk_sb = wqkv_sb[:, C : 2 * C]
            wv_sb = wqkv_sb[:, 2 * C : 3 * C]
            bqkv_sb = consts.tile([C, 3], F32)
            nc.scalar.dma_start(out=bqkv_sb, in_=bq_d)
            bq_sb = bqkv_sb[:, 0:1]
            bk_sb = bqkv_sb[:, 1:2]
            bv_sb = bqkv_sb[:, 2:3]
            wtc_sb = consts.tile([C, 1], BF16)
            nc.scalar.dma_start(out=wtc_sb, in_=wtc_d)

            # x transposed in by the DMA XBAR in 8 strided sweeps (pipelined
            # so logits can chase them). xc rows viewed as [(n k), c] with
            # row-stride 256B give layout xT[c, n, k] (k fastest on free).
            NP = 8
            PR = N // NP  # 128 tokens per piece
            xT = ph1big.tile([128, N, NCHUNK], BF16)  # [c, n, k] 4 MB
            xc_rows = xc.rearrange("n (k c) -> (n k) c", c=128)
            tp_insts = []
            for p in range(NP):
                hr = slice(p * PR * NCHUNK, (p + 1) * PR * NCHUNK)
                tp_insts.append(nc.sync.dma_start_transpose(
                    out=xT[:, p * PR : (p + 1) * PR, :],
                    in_=xc_rows[hr, :],
                ))

            # Wo hi/lo splits: one big DMA each, after the x transposes.
            for wo_sb, wo_src, dep in (
                (wo_hi, wo_hi_d, tp_insts[-2]),
                (wo_lo, wo_lo_d, tp_insts[-1]),
            ):
                wi = nc.gpsimd.dma_start(
                    out=wo_sb,
                    in_=wo_src.rearrange("(h p) o -> p h o", p=128),
                )
                add_dep_helper(wi.ins, dep.ins, sync=True, reason="wo after x")

            with tc.tile_pool(name="lg_psum", bufs=1, space="PSUM") as lg_psum:
                # logits[n] = sum_k xT[:, n, k] . wtc   (wtc = (Wl@Wq)/16),
                # one accumulation group per transpose piece so PE starts as
                # soon as the first piece lands.
                lg = lg_psum.tile([1, N], F32)
                negrow = mrows.tile([1, N], F32)
                for p in range(NP):
                    for k in range(NCHUNK):
                        nc.tensor.matmul(
                            lg[:, p * PR : (p + 1) * PR],
                            wtc_sb,
                            xT[:, p * PR : (p + 1) * PR, k],
                            start=(k == 0),
                            stop=(k == NCHUNK - 1),
                        )
                    # negate per piece: runs on DVE while later pieces land
                    nc.vector.tensor_scalar_mul(
                        negrow[:, p * PR : (p + 1) * PR],
                        lg[:, p * PR : (p + 1) * PR], -1.0,
                    )

                # ---- mask: softmax over tokens, snap all but 25 smallest
                # to 1. DVE runs the serial top-k; Act computes the softmax
                # normalization in parallel.
                m8 = mrows.tile([1, 8], F32)
                for _ in range(3):
                    nc.vector.max(out=m8, in_=negrow)
                    nc.vector.match_replace(
                        out=negrow, in_to_replace=m8, in_values=negrow,
                        imm_value=NEG_BIG,
                    )
                nc.vector.max(out=m8, in_=negrow)  # 25th largest of -L
                thrneg = mrows.tile([1, 1], F32)
                nc.vector.tensor_scalar_mul(thrneg, m8[:, 0:1], -1.0)
                urow = mrows.tile([1, N], F32)
                ssum = mrows.tile([1, 1], F32)
                nc.scalar.activation(
                    out=urow, in_=lg, func=Exp, accum_out=ssum
                )
                srecip = mrows.tile([1, 1], F32)
                nc.vector.reciprocal(srecip, ssum)
                # normalize on DVE: on the (in-order) Act queue this would
                # block all phase-1 Q/K moves behind the srecip wait
                smrow = mrows.tile([1, N], F32)
                nc.vector.tensor_scalar_mul(smrow, urow, srecip)
                sel = mrows.tile([1, N], U32)
                nc.vector.tensor_scalar(
                    sel, lg, thrneg, None, op0=mybir.AluOpType.is_gt
                )
                onesrow = mrows.tile([1, N], F32)
                nc.vector.memset(onesrow, 1.0)
                nc.vector.copy_predicated(smrow, sel, onesrow)
                # k = sum(mask) - 1024 (~ -25 + tiny): V-bias fold + dn fixup
                msum = mrows.tile([1, 1], F32)
                mdummy = mrows.tile([1, N], F32)
                nc.vector.tensor_scalar(
                    mdummy, smrow, 1.0, 0.0, op0=mybir.AluOpType.mult,
                    op1=mybir.AluOpType.add, accum_out=msum,
                )
                ksc = mrows.tile([1, 1], F32)
                nc.vector.tensor_scalar_add(ksc, msum, -float(N))
                nc.vector.tensor_scalar_mul(ksc32, ksc, 1.0 / A_SCALE)
                nc.vector.tensor_scalar_mul(kneg32, ksc, -1.0 / A_SCALE)

            # ---- Q/K/V projections, interleaved per head -----------------
            # Q/K produce transposed layouts [c', h, n]; V goes directly to
            # its PV-ready [token, channel] layout (x chunk as stationary).
            # V bias is handled via kbv + host bo fold; the V mask lives in
            # the exp bias (lnm_col), so V copies have no mask dependency.
            # Act paces the Q/K bias-moves; V copies go to DVE (idle once
            # the mask chain drains) except the last heads on Act.
            with tc.tile_pool(name="mc_psum", bufs=1, space="PSUM") as mc_psum:
                for h in range(HPC):
                    for jtg in range(2):
                        vp = v_psum.tile([128, 4, C], F32)
                        for dj in range(4):
                            jt = jtg * 4 + dj
                            nc.tensor.matmul(
                                vp[:, dj, :],
                                xT[:, jt * 128 : (jt + 1) * 128, h],
                                wv_sb,
                                start=True,
                                stop=True,
                            )
                        dst = vnat[:, h, jtg * 4 : (jtg + 1) * 4, :]
                        # V copies mostly on DVE (the Act queue gates phase 2
                        # when it carries late copies), but the first two
                        # heads' copies go to Act: their V matmuls finish
                        # before Act reaches them (no roadblock), and DVE's
                        # tail (chain + copies + mask cols) is now the gate
                        if h < 2:
                            nc.scalar.activation(out=dst, in_=vp, func=Ident)
                        else:
                            nc.vector.tensor_copy(dst, vp)
                    for w_sb, b_sb, dstT in ((wq_sb, bq_sb, qT), (wk_sb, bk_sb, kT)):
                        pp = mm_psum.tile([128, N], F32)
                        for half in range(2):
                            nc.tensor.matmul(
                                pp[:, half * 512 : (half + 1) * 512],
                                w_sb,
                                xT[:, half * 512 : (half + 1) * 512, h],
                                start=True,
                                stop=True,
                            )
                        nc.scalar.activation(
                            out=dstT[:, h, :], in_=pp, func=Ident, bias=b_sb
                        )
                    if h == 3:
                        # ---- mask row -> [128, 8] columns via tiny PE
                        # transposes (plus a [128,1] broadcast of k),
                        # replacing two DRAM bounces. Emitted mid-QKV: the
                        # PE is consumer-paced here (idle slots), smrow is
                        # ready by now, and scale_col/lnm_col stop gating
                        # phase 2's first exp.
                        mcol_ps = mc_psum.tile([128, 9], F32)
                        for t in range(8):
                            nc.tensor.transpose(
                                mcol_ps[:, t : t + 1],
                                smrow[:, t * 128 : (t + 1) * 128],
                                one_f32,
                            )
                        nc.tensor.matmul(
                            mcol_ps[:, 8:9], ones128_f32, ksc,
                            start=True, stop=True,
                        )
                        nc.vector.tensor_copy(mask_col, mcol_ps[:, 0:8])
                        nc.vector.tensor_scalar_mul(scale_col, mask_col, SCALE)
                        nc.vector.tensor_mul(kbv, mcol_ps[:, 8:9], bv_sb)
                    if h == 7:
                        # Ln deferred past all Q/K moves: it waits on
                        # mask_col, and anywhere earlier it stalls the
                        # in-order Act queue ahead of the remaining moves
                        nc.scalar.activation(
                            out=lnm_col, in_=mask_col, func=Ln
                        )

        # ================= phase 2: attention ==============================
        # Pool open order places dn/ot on the earliest-freed phase-1 banks.
        with (
            tc.tile_pool(name="pexp", bufs=2) as pexp_pool,
            tc.tile_pool(name="dvp", bufs=2) as dvp,
            tc.tile_pool(name="dn_psum", bufs=1, space="PSUM") as dn_psum,
            tc.tile_pool(name="ot_psum", bufs=1, space="PSUM") as ot_psum,
            tc.tile_pool(name="st_psum", bufs=2, space="PSUM") as st_psum,
        ):
            for h in range(HPC):
                ot = ot_psum.tile([128, N], F32, tag="ot")
                dn = dn_psum.tile([1, N], F32, tag="dn")
                pexp = pexp_pool.tile([128, 8, N], BF16, tag="pexp")

                def emit_pvdn(jt, ot=ot, dn=dn, pexp=pexp, h=h):
                    for half in range(2):
                        sl = slice(half * 512, (half + 1) * 512)
                        nc.tensor.matmul(
                            ot[:, sl],
                            vnat[:, h, jt, :],
                            pexp[:, jt, sl],
                            start=(jt == 0),
                            stop=(jt == 7),
                        )
                        nc.tensor.matmul(
                            dn[:, sl],
                            ones32,
                            pexp[:, jt, sl],
                            start=(jt == 0),
                            stop=(jt == 7 and h < HPC - 1),
                        )

                pending = None
                for jt in range(8):
                    st = st_psum.tile([128, N], F32, tag="st")
                    for half in range(2):
                        nc.tensor.matmul(
                            st[:, half * 512 : (half + 1) * 512],
                            kT[:, h, jt * 128 : (jt + 1) * 128],
                            qT[:, h, half * 512 : (half + 1) * 512],
                            start=True,
                            stop=True,
                        )
                    nc.scalar.activation(
                        out=pexp[:, jt, :], in_=st, func=Exp,
                        scale=scale_col[:, jt : jt + 1],
                        bias=lnm_col[:, jt : jt + 1],
                    )
                    if pending is not None:
                        emit_pvdn(pending)
                    pending = jt
                emit_pvdn(pending)
                if h == HPC - 1:
                    # fold -k/32 into dn in-PSUM (K=1 matmul): removes the
                    # DVE dnadj step from the to_out-gating drain chain
                    for half in range(2):
                        sl = slice(half * 512, (half + 1) * 512)
                        nc.tensor.matmul(
                            dn[:, sl], kneg32, onesrowN[:, sl],
                            start=False, stop=True,
                        )

                # drain: otsb = u + k*bv (DVE per-partition add, frees ot);
                # dn' holds sum_j m_j p_j / 32, true dn/32 = dn' - k/32 (the
                # 25 masked tokens have p ~= 1); tmp = otsb * (32/dn);
                # hi/lo fp8 split of tmp.
                otsb = dvp.tile([128, N], BF16, tag="otsb")
                if h < HPC - 1:
                    nc.vector.tensor_scalar(
                        otsb, ot, kbv, None, op0=mybir.AluOpType.add
                    )
                else:
                    # last head: Act is idle here; keep DVE free for the
                    # to_out-gating dnadj/recip/tmp/hi/lo chain
                    nc.scalar.activation(
                        out=otsb, in_=ot, func=Ident, bias=kbv
                    )
                if h < HPC - 1:
                    dnadj = dvp.tile([1, N], F32, tag="dnadj")
                    nc.vector.tensor_scalar(
                        dnadj, dn, ksc32, None, op0=mybir.AluOpType.subtract
                    )
                    rrow = dvp.tile([1, N], F32, tag="rrow")
                    nc.vector.reciprocal(rrow, dnadj)
                    w_i = nc.sync.dma_start(out=dscr[h, :], in_=rrow)
                    rb = dvp.tile([128, N], F32, tag="rb")
                    r_i = nc.sync.dma_start(
                        out=rb, in_=dscr[h, :].partition_broadcast(128)
                    )
                    add_dep_helper(r_i.ins, w_i.ins, sync=True,
                                   reason="recip RAW")
                    tmp = dvp.tile([128, N], BF16, tag="tmp")
                    nc.vector.tensor_mul(tmp, otsb, rb)
                    nc.vector.tensor_copy(outT_hi[:, h, :], tmp)
                    nc.vector.tensor_sub(
                        outT_lo[:, h, :], tmp, outT_hi[:, h, :]
                    )
                else:
                    # last head gates to_out: broadcast 1/dn across partitions
                    # with a K=1 matmul into the freed ot slot instead of the
                    # (slower) DRAM round-trip, and run the whole drain per
                    # half-column — the to_out DR matmuls consume hi/lo per
                    # half, so half 0's t3 instructions start ~2us earlier.
                    rrow_bf = dvp.tile([1, N], BF16, tag="rrowbf")
                    rb = ot_psum.tile([128, N], F32, tag="ot")
                    tmp = dvp.tile([128, N], BF16, tag="tmp")
                    for half in range(2):
                        sl = slice(half * 512, (half + 1) * 512)
                        with nc.allow_low_precision(
                            reason="1/dn broadcast row; 0.4% relative is fine"
                        ):
                            nc.vector.reciprocal(rrow_bf[:, sl], dn[:, sl])
                        nc.tensor.matmul(
                            rb[:, sl], ones128_bf, rrow_bf[:, sl],
                            start=True, stop=True,
                        )
                        nc.vector.tensor_mul(
                            tmp[:, sl], otsb[:, sl], rb[:, sl]
                        )
                        nc.vector.tensor_copy(
                            outT_hi[:, h, sl], tmp[:, sl]
                        )
                        nc.vector.tensor_sub(
                            outT_lo[:, h, sl], tmp[:, sl],
                            outT_hi[:, h, sl],
                        )

            # ============= phase 4: to_out partial (fp8 DoubleRow) =========
            # fo shares the st_psum slots (same shape) so Wo accumulation can
            # begin as soon as the last exp frees an ST slot.
            with tc.tile_pool(name="fout", bufs=3) as fout_pool:
                PRODUCTS = ((wo_hi, outT_hi), (wo_hi, outT_lo), (wo_lo, outT_hi))

                def finish_oc(oc, fo):
                    fout = fout_pool.tile([128, N], BF16)
                    eng = nc.sync if oc % 2 == 0 else nc.gpsimd
                    if oc < 15:
                        nc.vector.tensor_scalar_mul(
                            fout, fo, 1.0 / (WO_SCALE * A_SCALE)
                        )
                        eng.dma_start(
                            out=outT_d[oc * 128 : (oc + 1) * 128, :], in_=fout
                        )
                    else:
                        # final oc: drain per half so the tail DMA covers
                        # only 512 columns
                        for sh in range(2):
                            ssl = slice(sh * 512, (sh + 1) * 512)
                            nc.vector.tensor_scalar_mul(
                                fout[:, ssl], fo[:, ssl],
                                1.0 / (WO_SCALE * A_SCALE),
                            )
                            eng = nc.sync if sh == 0 else nc.gpsimd
                            eng.dma_start(
                                out=outT_d[oc * 128 : (oc + 1) * 128, ssl],
                                in_=fout[:, ssl],
                            )

                pending_oc = None
                for oc in range(16):
                    fo = st_psum.tile([128, N], F32, tag="st")
                    for half in range(2):
                        sl = slice(half * 512, (half + 1) * 512)
                        # t=3 (heads 6,7) last in every product so the first
                        # 9 instructions don't wait on head 7's drain chain
                        steps = [(w, a, t) for t in (0, 1, 2)
                                 for (w, a) in PRODUCTS]
                        steps += [(w, a, 3) for (w, a) in PRODUCTS]
                        for i, (wsp, asp, t) in enumerate(steps):
                            nc.tensor.matmul(
                                fo[:, sl],
                                wsp[:, 2 * t : 2 * t + 2,
                                    oc * 128 : (oc + 1) * 128],
                                asp[:, 2 * t : 2 * t + 2, sl],
                                start=(i == 0),
                                stop=(i == len(steps) - 1),
                                perf_mode=DR,
                            )
                    if pending_oc is not None:
                        finish_oc(*pending_oc)
                    pending_oc = (oc, fo)
                finish_oc(*pending_oc)


_CACHE = {}


def _get_module():
    if "nc" in _CACHE:
        return _CACHE["nc"]
    nc = bacc.Bacc("TRN2", target_bir_lowering=False, debug=False, num_devices=8)
    xc = nc.dram_tensor("xc", (N, D), BF16, kind="ExternalInput").ap()
    wq_d = nc.dram_tensor("wqkvT", (C, 3 * C), BF16, kind="ExternalInput").ap()
    bq_d = nc.dram_tensor("bqkv", (C, 3), F32, kind="ExternalInput").ap()
    wtc_d = nc.dram_tensor("wtc", (C, 1), BF16, kind="ExternalInput").ap()
    wo_hi_d = nc.dram_tensor("woHi", (HPC * C, D), FP8, kind="ExternalInput").ap()
    wo_lo_d = nc.dram_tensor("woLo", (HPC * C, D), FP8, kind="ExternalInput").ap()
    outT_d = nc.dram_tensor("outT", (D, N), BF16, kind="ExternalOutput").ap()

    with tile.TileContext(nc) as tc:
        _body(tc, xc, wq_d, bq_d, wtc_d, wo_hi_d, wo_lo_d, outT_d)
    nc.compile()
    _CACHE["nc"] = nc
    return nc


def make_in_maps(x, Wq, bq, Wk, bk, Wv, bv, Wl, bl, Wo, bo):
    bf16 = ml_dtypes.bfloat16
    fp8 = ml_dtypes.float8_e4m3
    x = np.ascontiguousarray(np.asarray(x, np.float32))
    Wq = np.asarray(Wq, np.float32)
    Wk = np.asarray(Wk, np.float32)
    Wv = np.asarray(Wv, np.float32)
    Wl = np.asarray(Wl, np.float32)
    Wo = np.asarray(Wo, np.float32)
    we = (Wl[0] @ Wq) / float(NCHUNK)  # (128,) logits weight per chunk
    common = {
        "wqkvT": np.ascontiguousarray(
            np.concatenate([Wq.T, Wk.T, Wv.T], axis=1)
        ).astype(bf16),
        "bqkv": np.ascontiguousarray(np.stack(
            [np.asarray(bq, np.float32), np.asarray(bk, np.float32),
             np.asarray(bv, np.float32)], axis=1
        )),
        "wtc": we.astype(bf16).reshape(C, 1),
    }
    woT = np.ascontiguousarray(Wo.T) * WO_SCALE  # (d, o), prescaled
    in_maps = []
    halves = {}
    for g in range(2):
        wh = woT[g * 1024 : (g + 1) * 1024, :]
        hi = wh.astype(fp8)
        lo = (wh - hi.astype(np.float32)).astype(fp8)
        halves[g] = (np.ascontiguousarray(hi), np.ascontiguousarray(lo))
    for core in range(8):
        b, g = divmod(core, 2)
        xb = x[b]
        xcore = xb if g == 0 else np.concatenate(
            [xb[:, 1024:], xb[:, :1024]], axis=1
        )
        in_maps.append({
            "xc": np.ascontiguousarray(xcore.astype(bf16)),
            "woHi": halves[g][0],
            "woLo": halves[g][1],
            **common,
        })
    return in_maps


def run_spmd(in_maps, trace=False, **kw):
    nc = _get_module()
    return bass_utils.run_bass_kernel_spmd(
        nc, in_maps, core_ids=list(range(8)), trace=trace, **kw
    )


def gather(results, Wo, bv, bo):
    Wo = np.asarray(Wo, np.float32)
    bv = np.asarray(bv, np.float32)
    bo = np.asarray(bo, np.float32)
    # a = a_tilde + bv per head-channel: fold bv through Wo into the bias.
    bo_eff = bo + np.tile(bv, NCHUNK) @ Wo.T
    out = np.empty((B, N, D), np.float32)
    for b in range(B):
        p0 = results[2 * b]["outT"].astype(np.float32).T
        p1 = results[2 * b + 1]["outT"].astype(np.float32).T
        out[b] = p0 + p1 + bo_eff
    return out


def kernel(x, Wq, bq, Wk, bk, Wv, bv, Wl, bl, Wo, bo, stage=None, **_unused):
    in_maps = make_in_maps(x, Wq, bq, Wk, bk, Wv, bv, Wl, bl, Wo, bo)
    try:
        res = run_spmd(in_maps)
    except Exception:
        # transient device/runtime hiccup: retry once after a short pause
        import time as _time

        _time.sleep(2.0)
        res = run_spmd(in_maps)
    return gather(res.results, Wo, bv, bo)
